# revision 10
# baseline (speedup 1.0000x reference)
"""Trainium2 Bass kernel for nn_CharStemmer (bi-LSTM encoder + LSTM decoder).

Sharding: data-parallel over batch (B=128) across 8 cores, 16 sequences per
core; all weights replicated. Inside each core:
  - embedding lookup as one-hot matmul
  - input-side gate GEMMs (xg = emb @ w_ih^T + b) batched over all timesteps
  - the three recurrences run step-by-step; per step the stationary matmul
    operand is h^T (tiny) and w_hh^T streams through the PE in bf16
  - h is computed batch-major [16, 1024] for full-width elementwise, then
    PE-transposed into the hidden-major history h^T used as next-step lhsT
  - final vocab projection from the stored h^T history.
"""

import os
import sys

for _p in ("/opt/trn_rl_repo", "/root/.axon_site/_ro/trn_rl_repo"):
    if os.path.isdir(_p) and _p not in sys.path:
        sys.path.insert(0, _p)

from contextlib import ExitStack

import ml_dtypes
import numpy as np

import concourse.bass as bass
import concourse.tile as tile
from concourse import bacc, mybir
from concourse.bass_utils import run_bass_kernel_spmd

S, B, V, E, H = 128, 128, 61, 512, 1024
NCORES = 8
BL = B // NCORES          # 16 sequences per core
G4 = 4 * H                # 4096 gate columns
VP = 64                   # vocab padded to 64 partitions
BF16 = mybir.dt.bfloat16
F32 = mybir.dt.float32
AF = mybir.ActivationFunctionType
bf16_np = ml_dtypes.bfloat16

# gate quarters in permuted order: q0=i, q1=g, q2=f, q3=o


def _build(nc, n_steps):
    TOK = n_steps * BL

    def din(name, shape, dt):
        return nc.dram_tensor(name, list(shape), dt, kind="ExternalInput").ap()

    onehot_d = din("onehot", [VP, TOK], BF16)
    embp_d = din("embp", [VP, E], BF16)
    wih_f_d = din("wih_f", [4, 128, G4], BF16)
    wih_b_d = din("wih_b", [4, 128, G4], BF16)
    whh_f_d = din("whh_f", [8, 128, G4], BF16)
    whh_b_d = din("whh_b", [8, 128, G4], BF16)
    wih_d_d = din("wih_d", [16, 128, G4], BF16)
    whh_d_d = din("whh_d", [8, 128, G4], BF16)
    bias_f_d = din("bias_f", [128, G4], F32)
    bias_b_d = din("bias_b", [128, G4], F32)
    bias_d_d = din("bias_d", [128, G4], F32)
    decb0_d = din("decb0", [BL, G4], F32)   # decoder step-0 gates (bias only)
    wout_d = din("wout", [8, 128, V], BF16)
    outb_d = din("outb", [V, 1], F32)
    ident_d = din("ident", [BL, BL], BF16)
    pred_d = nc.dram_tensor("pred", [V, TOK], F32, kind="ExternalOutput").ap()

    with ExitStack() as ctx:
        tc = ctx.enter_context(tile.TileContext(nc))
        dram = ctx.enter_context(tc.tile_pool(name="dram", bufs=1, space="DRAM"))
        xg_f = dram.tile([TOK, G4], F32, tag="xgf")
        xg_b = dram.tile([TOK, G4], F32, tag="xgb")
        ug_d = dram.tile([TOK, G4], F32, tag="ugd")

        persist = ctx.enter_context(tc.tile_pool(name="persist", bufs=1))
        hist = ctx.enter_context(tc.tile_pool(name="hist", bufs=2))

        ident_sb = persist.tile([BL, BL], BF16, tag="ident")
        nc.sync.dma_start(ident_sb[:], ident_d[:])

        # ---------------- phase E+X: embedding + input-side gate GEMMs -------
        n_tok_chunks = TOK // 512 if TOK >= 512 else 1
        tok_chunk = min(TOK, 512)
        n_tok_blocks = TOK // 128 if TOK >= 128 else 1
        tok_block = min(TOK, 128)

        with (
            tc.tile_pool(name="px", bufs=1) as px,
            tc.tile_pool(name="px_ps", bufs=8, space="PSUM") as px_ps,
            tc.tile_pool(name="px_ev", bufs=4) as px_ev,
        ):
            oh_sb = px.tile([VP, TOK], BF16, tag="oh")
            for j in range(TOK // 2048 if TOK >= 2048 else 1):
                w = min(TOK, 2048)
                nc.sync.dma_start(oh_sb[:, j * w:(j + 1) * w],
                                  onehot_d[:, j * w:(j + 1) * w])
            embp_sb = px.tile([VP, E], BF16, tag="embp")
            nc.sync.dma_start(embp_sb[:], embp_d[:])
            embT_sb = px.tile([128, 4 * TOK], BF16, tag="embT")

            # emb^T[e_chunk, tok] = embp.T @ onehot
            for m in range(4):
                for n in range(n_tok_chunks):
                    ps = px_ps.tile([128, tok_chunk], F32, tag="ps")
                    nc.tensor.matmul(
                        ps[:], embp_sb[:, m * 128:(m + 1) * 128],
                        oh_sb[:, n * tok_chunk:(n + 1) * tok_chunk],
                        start=True, stop=True)
                    nc.vector.tensor_copy(
                        embT_sb[:, m * TOK + n * tok_chunk:
                                m * TOK + (n + 1) * tok_chunk], ps[:])

            # xg = emb @ w_ih^T + b   (token-major [TOK, G4], f32, to DRAM)
            for wih_src, bias_src, xg_dst in (
                (wih_f_d, bias_f_d, xg_f),
                (wih_b_d, bias_b_d, xg_b),
            ):
                wih_sb = px.tile([128, 4, G4], BF16, tag="wih")
                for k in range(4):
                    nc.sync.dma_start(wih_sb[:, k, :], wih_src[k])
                bias_sb = px.tile([128, G4], F32, tag="bias")
                nc.sync.dma_start(bias_sb[:], bias_src[:])
                for m in range(n_tok_blocks):
                    for n in range(8):
                        ps = px_ps.tile([tok_block, 512], F32, tag="ps")
                        for k in range(4):
                            nc.tensor.matmul(
                                ps[:],
                                embT_sb[:, k * TOK + m * tok_block:
                                        k * TOK + (m + 1) * tok_block],
                                wih_sb[:, k, n * 512:(n + 1) * 512],
                                start=(k == 0), stop=(k == 3))
                        ev = px_ev.tile([tok_block, 512], F32, tag="ev")
                        nc.vector.tensor_add(
                            ev[:], ps[:], bias_sb[:tok_block, n * 512:(n + 1) * 512])
                        nc.sync.dma_start(
                            xg_dst[m * tok_block:(m + 1) * tok_block,
                                   n * 512:(n + 1) * 512], ev[:])

        # ---------------- recurrence helper ---------------------------------
        def recurrence(name, whh_src, xg_src, hT, reverse, dec_first=None):
            """One LSTM over n_steps. hT: [128, 8, TOK] bf16 history tile.
            xg_src rows are read at time t (dec: t-1, with dec_first for t=0)."""
            whh_sb = persist.tile([128, 8, G4], BF16, tag="whh")
            for k in range(8):
                nc.sync.dma_start(whh_sb[:, k, :], whh_src[k])

            with (
                tc.tile_pool(name=name + "_ew", bufs=1) as ew,
                tc.tile_pool(name=name + "_xq", bufs=4) as xqp,
                tc.tile_pool(name=name + "_q", bufs=3, space="PSUM") as qps,
                tc.tile_pool(name=name + "_tr", bufs=2, space="PSUM") as trps,
            ):
                c_sb = ew.tile([BL, H], F32, tag="c")
                for s in range(n_steps):
                    t = (n_steps - 1 - s) if reverse else s
                    tprev = (n_steps - s) if reverse else (s - 1)
                    gq = []
                    for q in range(4):
                        xq = xqp.tile([BL, H], F32, tag="xq")
                        if dec_first is not None and t == 0:
                            nc.sync.dma_start(
                                xq[:], dec_first[:, q * H:(q + 1) * H])
                        else:
                            tsrc = (t - 1) if dec_first is not None else t
                            nc.sync.dma_start(
                                xq[:], xg_src[tsrc * BL:(tsrc + 1) * BL,
                                              q * H:(q + 1) * H])
                        if s == 0:
                            gq.append(xq)
                            continue
                        ps = qps.tile([BL, H], F32, tag="q")
                        for nn in range(2):
                            for k in range(8):
                                nc.tensor.matmul(
                                    ps[:, nn * 512:(nn + 1) * 512],
                                    hT[:, k, tprev * BL:(tprev + 1) * BL],
                                    whh_sb[:, k, q * H + nn * 512:
                                           q * H + (nn + 1) * 512],
                                    start=(k == 0), stop=(k == 7))
                        g = ew.tile([BL, H], F32, tag="g", bufs=2)
                        nc.vector.tensor_add(g[:], ps[:], xq[:])
                        gq.append(g)

                    si = ew.tile([BL, H], F32, tag="si", bufs=1)
                    nc.scalar.activation(si[:], gq[0][:], AF.Sigmoid)
                    tg = ew.tile([BL, H], F32, tag="tg", bufs=1)
                    nc.scalar.activation(tg[:], gq[1][:], AF.Tanh)
                    a = ew.tile([BL, H], F32, tag="a", bufs=1)
                    nc.vector.tensor_mul(a[:], si[:], tg[:])
                    if s == 0:
                        nc.vector.tensor_copy(c_sb[:], a[:])
                    else:
                        sf = ew.tile([BL, H], F32, tag="sf", bufs=1)
                        nc.scalar.activation(sf[:], gq[2][:], AF.Sigmoid)
                        t1 = ew.tile([BL, H], F32, tag="t1", bufs=1)
                        nc.vector.tensor_mul(t1[:], sf[:], c_sb[:])
                        nc.vector.tensor_add(c_sb[:], t1[:], a[:])
                    tc_ = ew.tile([BL, H], F32, tag="tc", bufs=1)
                    nc.scalar.activation(tc_[:], c_sb[:], AF.Tanh)
                    so = ew.tile([BL, H], F32, tag="so", bufs=1)
                    nc.scalar.activation(so[:], gq[3][:], AF.Sigmoid)
                    h = ew.tile([BL, H], BF16, tag="h", bufs=2)
                    nc.vector.tensor_mul(h[:], so[:], tc_[:])

                    trb = trps.tile([128, 8 * BL], BF16, tag="tr")
                    for j in range(8):
                        nc.tensor.transpose(
                            trb[:, j * BL:(j + 1) * BL],
                            h[:, j * 128:(j + 1) * 128], ident_sb[:])
                    trb_r = trb.rearrange("p (k b) -> p k b", k=8)
                    nc.vector.tensor_copy(
                        hT[:, :, t * BL:(t + 1) * BL], trb_r[:, :, :])

        # ---------------- phase R1: encoder fwd + bwd ------------------------
        fwd_hT = hist.tile([128, 8, TOK], BF16, tag="hist")
        recurrence("rf", whh_f_d, xg_f, fwd_hT, reverse=False)
        bwd_hT = hist.tile([128, 8, TOK], BF16, tag="hist")
        recurrence("rb", whh_b_d, xg_b, bwd_hT, reverse=True)

        # ---------------- phase U: decoder input GEMM ------------------------
        # ug[tok] = encoded[tok] @ dec_w_ih^T + dec_b   (unshifted; read at t-1)
        with (
            tc.tile_pool(name="pu", bufs=1) as pu,
            tc.tile_pool(name="pu_w", bufs=3) as puw,
            tc.tile_pool(name="pu_ps", bufs=8, space="PSUM") as pups,
            tc.tile_pool(name="pu_ev", bufs=4) as puev,
        ):
            bias_sb = pu.tile([128, G4], F32, tag="biasd")
            nc.sync.dma_start(bias_sb[:], bias_d_d[:])
            n_mg = max(1, n_tok_blocks // 8)
            mg_w = min(8, n_tok_blocks)
            for n in range(8):
                for mg in range(n_mg):
                    pss = [pups.tile([tok_block, 512], F32, tag="ps",
                                     name=f"ps_{n}_{mg}_{i}")
                           for i in range(mg_w)]
                    for k in range(16):
                        wt = puw.tile([128, 512], BF16, tag="wt")
                        nc.sync.dma_start(wt[:], wih_d_d[k, :, n * 512:(n + 1) * 512])
                        src = fwd_hT if k < 8 else bwd_hT
                        for m in range(mg_w):
                            mb = mg * mg_w + m
                            nc.tensor.matmul(
                                pss[m][:],
                                src[:, k % 8, mb * tok_block:(mb + 1) * tok_block],
                                wt[:], start=(k == 0), stop=(k == 15))
                    for m in range(mg_w):
                        mb = mg * mg_w + m
                        ev = puev.tile([tok_block, 512], F32, tag="ev")
                        nc.vector.tensor_add(
                            ev[:], pss[m][:],
                            bias_sb[:tok_block, n * 512:(n + 1) * 512])
                        nc.sync.dma_start(
                            ug_d[mb * tok_block:(mb + 1) * tok_block,
                                 n * 512:(n + 1) * 512], ev[:])

        # ---------------- phase R2: decoder recurrence -----------------------
        dec_hT = hist.tile([128, 8, TOK], BF16, tag="hist")
        recurrence("rd", whh_d_d, ug_d, dec_hT, reverse=False, dec_first=decb0_d)

        # ---------------- phase P: vocab projection --------------------------
        with (
            tc.tile_pool(name="pp", bufs=1) as pp,
            tc.tile_pool(name="pp_ps", bufs=2, space="PSUM") as ppps,
            tc.tile_pool(name="pp_ev", bufs=2) as ppev,
        ):
            wout_sb = pp.tile([128, 8, V], BF16, tag="wout")
            for k in range(8):
                nc.sync.dma_start(wout_sb[:, k, :], wout_d[k])
            outb_sb = pp.tile([V, 1], F32, tag="outb")
            nc.sync.dma_start(outb_sb[:], outb_d[:])
            for n in range(n_tok_chunks):
                ps = ppps.tile([V, tok_chunk], F32, tag="ps")
                for k in range(8):
                    nc.tensor.matmul(
                        ps[:], wout_sb[:, k, :],
                        dec_hT[:, k, n * tok_chunk:(n + 1) * tok_chunk],
                        start=(k == 0), stop=(k == 7))
                ev = ppev.tile([V, tok_chunk], F32, tag="ev")
                nc.vector.tensor_scalar_add(ev[:], ps[:], outb_sb[:])
                nc.sync.dma_start(pred_d[:, n * tok_chunk:(n + 1) * tok_chunk], ev[:])

    return nc


_CACHE = {}


def _get_nc(n_steps):
    if n_steps not in _CACHE:
        nc = bacc.Bacc("TRN2", target_bir_lowering=False, debug=False)
        _build(nc, n_steps)
        nc.compile()
        _CACHE[n_steps] = nc
    return _CACHE[n_steps]


def _gate_perm():
    r = np.arange(G4)
    return np.concatenate([r[0:H], r[2 * H:3 * H], r[H:2 * H], r[3 * H:4 * H]])


def _prep_shared(embedding, enc_w_ih_f, enc_w_hh_f, enc_b_f, enc_w_ih_b,
                 enc_w_hh_b, enc_b_b, dec_w_ih, dec_w_hh, dec_b, out_w, out_b):
    p = _gate_perm()

    def wT(w, kt):
        return np.ascontiguousarray(
            w[p].T.reshape(kt, 128, G4).astype(bf16_np))

    embp = np.zeros((VP, E), np.float32)
    embp[:V] = embedding
    shared = {
        "embp": embp.astype(bf16_np),
        "wih_f": wT(enc_w_ih_f, 4),
        "wih_b": wT(enc_w_ih_b, 4),
        "whh_f": wT(enc_w_hh_f, 8),
        "whh_b": wT(enc_w_hh_b, 8),
        "wih_d": wT(dec_w_ih, 16),
        "whh_d": wT(dec_w_hh, 8),
        "bias_f": np.broadcast_to(enc_b_f[p], (128, G4)).astype(np.float32).copy(),
        "bias_b": np.broadcast_to(enc_b_b[p], (128, G4)).astype(np.float32).copy(),
        "bias_d": np.broadcast_to(dec_b[p], (128, G4)).astype(np.float32).copy(),
        "decb0": np.broadcast_to(dec_b[p], (BL, G4)).astype(np.float32).copy(),
        "wout": np.ascontiguousarray(
            out_w.T.reshape(8, 128, V).astype(bf16_np)),
        "outb": out_b.reshape(V, 1).astype(np.float32),
        "ident": np.eye(BL, dtype=bf16_np),
    }
    return shared


def _in_maps(inputs, n_steps):
    input_seq = np.asarray(inputs["input_seq"]).astype(np.int64)
    shared = _prep_shared(
        *[np.asarray(inputs[k], np.float32) for k in (
            "embedding", "enc_w_ih_f", "enc_w_hh_f", "enc_b_f",
            "enc_w_ih_b", "enc_w_hh_b", "enc_b_b",
            "dec_w_ih", "dec_w_hh", "dec_b", "out_w", "out_b")])
    TOK = n_steps * BL
    in_maps = []
    for c in range(NCORES):
        idx = input_seq[:n_steps, c * BL:(c + 1) * BL]  # [n_steps, BL]
        oh = np.zeros((VP, TOK), np.float32)
        cols = np.arange(TOK)
        oh[idx.reshape(-1), cols] = 1.0
        m = dict(shared)
        m["onehot"] = oh.astype(bf16_np)
        in_maps.append(m)
    return in_maps


def _assemble(res, n_steps):
    outs = []
    for c in range(NCORES):
        pr = res.results[c]["pred"]            # [V, TOK]
        outs.append(pr.reshape(V, n_steps, BL).transpose(1, 2, 0))
    return np.concatenate(outs, axis=1).astype(np.float32)  # [n_steps, B, V]


def _run(inputs, n_steps):
    in_maps = _in_maps(inputs, n_steps)
    nc = _get_nc(n_steps)
    res = run_bass_kernel_spmd(nc, in_maps, core_ids=list(range(NCORES)))
    return _assemble(res, n_steps)


def _run_traced(inputs, n_steps):
    in_maps = _in_maps(inputs, n_steps)
    nc = _get_nc(n_steps)
    res = run_bass_kernel_spmd(nc, in_maps, core_ids=list(range(NCORES)),
                               trace=True)
    return _assemble(res, n_steps), res


def kernel(**inputs):
    return _run(inputs, S)


# revision 11
# speedup vs baseline: 8.3622x; 8.3622x over previous
"""Trainium2 Bass kernel for nn_CharStemmer (bi-LSTM encoder + LSTM decoder).

Sharding: data-parallel over batch (B=128) across 8 cores, 16 sequences per
core; all weights replicated. Inside each core:
  - embedding lookup as one-hot matmul
  - input-side gate GEMMs (xg = emb @ w_ih^T + b) batched over all timesteps
  - the three recurrences run step-by-step; per step the stationary matmul
    operand is h^T (tiny) and w_hh^T streams through the PE in bf16
  - h is computed batch-major [16, 1024] for full-width elementwise, then
    PE-transposed into the hidden-major history h^T used as next-step lhsT
  - final vocab projection from the stored h^T history.
"""

import os
import sys

for _p in ("/opt/trn_rl_repo", "/root/.axon_site/_ro/trn_rl_repo"):
    if os.path.isdir(_p) and _p not in sys.path:
        sys.path.insert(0, _p)

from contextlib import ExitStack

import ml_dtypes
import numpy as np

import concourse.bass as bass
import concourse.tile as tile
from concourse import bacc, mybir
from concourse.bass_utils import run_bass_kernel_spmd

S, B, V, E, H = 128, 128, 61, 512, 1024
NCORES = 8
BL = B // NCORES          # 16 sequences per core
G4 = 4 * H                # 4096 gate columns
VP = 64                   # vocab padded to 64 partitions
BF16 = mybir.dt.bfloat16
F32 = mybir.dt.float32
AF = mybir.ActivationFunctionType
bf16_np = ml_dtypes.bfloat16

# gate quarters in permuted order: q0=i, q1=g, q2=f, q3=o


def _build(nc, n_steps):
    TOK = n_steps * BL

    def din(name, shape, dt):
        return nc.dram_tensor(name, list(shape), dt, kind="ExternalInput").ap()

    onehot_d = din("onehot", [VP, TOK], BF16)
    embp_d = din("embp", [VP, E], BF16)
    wih_f_d = din("wih_f", [4, 128, G4], BF16)
    wih_b_d = din("wih_b", [4, 128, G4], BF16)
    whh_f_d = din("whh_f", [8, 128, G4], BF16)
    whh_b_d = din("whh_b", [8, 128, G4], BF16)
    wih_d_d = din("wih_d", [16, 128, G4], BF16)
    whh_d_d = din("whh_d", [8, 128, G4], BF16)
    bias_f_d = din("bias_f", [128, G4], F32)
    bias_b_d = din("bias_b", [128, G4], F32)
    bias_d_d = din("bias_d", [128, G4], F32)
    decb0_d = din("decb0", [BL, G4], F32)   # decoder step-0 gates (bias only)
    wout_d = din("wout", [8, 128, V], BF16)
    outb_d = din("outb", [V, 1], F32)
    ident_d = din("ident", [BL, BL], BF16)
    pred_d = nc.dram_tensor("pred", [V, TOK], F32, kind="ExternalOutput").ap()

    with ExitStack() as ctx:
        tc = ctx.enter_context(tile.TileContext(nc))
        dram = ctx.enter_context(tc.tile_pool(name="dram", bufs=1, space="DRAM"))
        xg_f = dram.tile([TOK, G4], F32, tag="xgf")
        xg_b = dram.tile([TOK, G4], F32, tag="xgb")
        ug_d = dram.tile([TOK, G4], F32, tag="ugd")

        persist = ctx.enter_context(tc.tile_pool(name="persist", bufs=1))
        hist = ctx.enter_context(tc.tile_pool(name="hist", bufs=2))

        ident_sb = persist.tile([BL, BL], BF16, tag="ident")
        nc.sync.dma_start(ident_sb[:], ident_d[:])

        # ---------------- phase E+X: embedding + input-side gate GEMMs -------
        n_tok_chunks = TOK // 512 if TOK >= 512 else 1
        tok_chunk = min(TOK, 512)
        n_tok_blocks = TOK // 128 if TOK >= 128 else 1
        tok_block = min(TOK, 128)

        with (
            tc.tile_pool(name="px", bufs=1) as px,
            tc.tile_pool(name="px_ps", bufs=8, space="PSUM") as px_ps,
            tc.tile_pool(name="px_ev", bufs=4) as px_ev,
        ):
            oh_sb = px.tile([VP, TOK], BF16, tag="oh")
            for j in range(TOK // 2048 if TOK >= 2048 else 1):
                w = min(TOK, 2048)
                nc.sync.dma_start(oh_sb[:, j * w:(j + 1) * w],
                                  onehot_d[:, j * w:(j + 1) * w])
            embp_sb = px.tile([VP, E], BF16, tag="embp")
            nc.sync.dma_start(embp_sb[:], embp_d[:])
            embT_sb = px.tile([128, 4 * TOK], BF16, tag="embT")

            # emb^T[e_chunk, tok] = embp.T @ onehot
            for m in range(4):
                for n in range(n_tok_chunks):
                    ps = px_ps.tile([128, tok_chunk], F32, tag="ps")
                    nc.tensor.matmul(
                        ps[:], embp_sb[:, m * 128:(m + 1) * 128],
                        oh_sb[:, n * tok_chunk:(n + 1) * tok_chunk],
                        start=True, stop=True)
                    nc.vector.tensor_copy(
                        embT_sb[:, m * TOK + n * tok_chunk:
                                m * TOK + (n + 1) * tok_chunk], ps[:])

            # xg = emb @ w_ih^T + b   (token-major [TOK, G4], f32, to DRAM)
            for wih_src, bias_src, xg_dst in (
                (wih_f_d, bias_f_d, xg_f),
                (wih_b_d, bias_b_d, xg_b),
            ):
                wih_sb = px.tile([128, 4, G4], BF16, tag="wih")
                for k in range(4):
                    nc.sync.dma_start(wih_sb[:, k, :], wih_src[k])
                bias_sb = px.tile([128, G4], F32, tag="bias")
                nc.sync.dma_start(bias_sb[:], bias_src[:])
                for m in range(n_tok_blocks):
                    for n in range(8):
                        ps = px_ps.tile([tok_block, 512], F32, tag="ps")
                        for k in range(4):
                            nc.tensor.matmul(
                                ps[:],
                                embT_sb[:, k * TOK + m * tok_block:
                                        k * TOK + (m + 1) * tok_block],
                                wih_sb[:, k, n * 512:(n + 1) * 512],
                                start=(k == 0), stop=(k == 3))
                        ev = px_ev.tile([tok_block, 512], F32, tag="ev")
                        nc.vector.tensor_add(
                            ev[:], ps[:], bias_sb[:tok_block, n * 512:(n + 1) * 512])
                        nc.sync.dma_start(
                            xg_dst[m * tok_block:(m + 1) * tok_block,
                                   n * 512:(n + 1) * 512], ev[:])

        # ---------------- recurrence helper ---------------------------------
        def recurrence(name, whh_src, xg_src, hT, reverse, dec_first=None):
            """One LSTM over n_steps. hT: [128, 8, TOK] bf16 history tile.
            xg_src rows are read at time t (dec: t-1, with dec_first for t=0)."""
            whh_sb = persist.tile([128, 8, G4], BF16, tag="whh")
            for k in range(8):
                nc.sync.dma_start(whh_sb[:, k, :], whh_src[k])

            with (
                tc.tile_pool(name=name + "_ew", bufs=1) as ew,
                tc.tile_pool(name=name + "_xq", bufs=4) as xqp,
                tc.tile_pool(name=name + "_q", bufs=3, space="PSUM") as qps,
                tc.tile_pool(name=name + "_tr", bufs=2, space="PSUM") as trps,
            ):
                c_sb = ew.tile([BL, H], F32, tag="c")
                for s in range(n_steps):
                    t = (n_steps - 1 - s) if reverse else s
                    tprev = (n_steps - s) if reverse else (s - 1)
                    gq = []
                    for q in range(4):
                        xq = xqp.tile([BL, H], F32, tag="xq")
                        if dec_first is not None and t == 0:
                            nc.sync.dma_start(
                                xq[:], dec_first[:, q * H:(q + 1) * H])
                        else:
                            tsrc = (t - 1) if dec_first is not None else t
                            nc.sync.dma_start(
                                xq[:], xg_src[tsrc * BL:(tsrc + 1) * BL,
                                              q * H:(q + 1) * H])
                        if s == 0:
                            gq.append(xq)
                            continue
                        ps = qps.tile([BL, H], F32, tag="q")
                        for nn in range(2):
                            for k in range(8):
                                nc.tensor.matmul(
                                    ps[:, nn * 512:(nn + 1) * 512],
                                    hT[:, k, tprev * BL:(tprev + 1) * BL],
                                    whh_sb[:, k, q * H + nn * 512:
                                           q * H + (nn + 1) * 512],
                                    start=(k == 0), stop=(k == 7))
                        g = ew.tile([BL, H], F32, tag="g", bufs=2)
                        nc.vector.tensor_add(g[:], ps[:], xq[:])
                        gq.append(g)

                    si = ew.tile([BL, H], F32, tag="si", bufs=1)
                    nc.scalar.activation(si[:], gq[0][:], AF.Sigmoid)
                    tg = ew.tile([BL, H], F32, tag="tg", bufs=1)
                    nc.scalar.activation(tg[:], gq[1][:], AF.Tanh)
                    a = ew.tile([BL, H], F32, tag="a", bufs=1)
                    nc.vector.tensor_mul(a[:], si[:], tg[:])
                    if s == 0:
                        nc.vector.tensor_copy(c_sb[:], a[:])
                    else:
                        sf = ew.tile([BL, H], F32, tag="sf", bufs=1)
                        nc.scalar.activation(sf[:], gq[2][:], AF.Sigmoid)
                        t1 = ew.tile([BL, H], F32, tag="t1", bufs=1)
                        nc.vector.tensor_mul(t1[:], sf[:], c_sb[:])
                        nc.vector.tensor_add(c_sb[:], t1[:], a[:])
                    tc_ = ew.tile([BL, H], F32, tag="tc", bufs=1)
                    nc.scalar.activation(tc_[:], c_sb[:], AF.Tanh)
                    so = ew.tile([BL, H], F32, tag="so", bufs=1)
                    nc.scalar.activation(so[:], gq[3][:], AF.Sigmoid)
                    h = ew.tile([BL, H], BF16, tag="h", bufs=2)
                    nc.vector.tensor_mul(h[:], so[:], tc_[:])

                    trb = trps.tile([128, 8 * BL], BF16, tag="tr")
                    for j in range(8):
                        nc.tensor.transpose(
                            trb[:, j * BL:(j + 1) * BL],
                            h[:, j * 128:(j + 1) * 128], ident_sb[:])
                    trb_r = trb.rearrange("p (k b) -> p k b", k=8)
                    nc.vector.tensor_copy(
                        hT[:, :, t * BL:(t + 1) * BL], trb_r[:, :, :])

        # ---------------- phase R1: encoder fwd + bwd ------------------------
        fwd_hT = hist.tile([128, 8, TOK], BF16, tag="hist")
        recurrence("rf", whh_f_d, xg_f, fwd_hT, reverse=False)
        bwd_hT = hist.tile([128, 8, TOK], BF16, tag="hist")
        recurrence("rb", whh_b_d, xg_b, bwd_hT, reverse=True)

        # ---------------- phase U: decoder input GEMM ------------------------
        # ug[tok] = encoded[tok] @ dec_w_ih^T + dec_b   (unshifted; read at t-1)
        with (
            tc.tile_pool(name="pu", bufs=1) as pu,
            tc.tile_pool(name="pu_w", bufs=3) as puw,
            tc.tile_pool(name="pu_ps", bufs=8, space="PSUM") as pups,
            tc.tile_pool(name="pu_ev", bufs=4) as puev,
        ):
            bias_sb = pu.tile([128, G4], F32, tag="biasd")
            nc.sync.dma_start(bias_sb[:], bias_d_d[:])
            n_mg = max(1, n_tok_blocks // 8)
            mg_w = min(8, n_tok_blocks)
            for n in range(8):
                for mg in range(n_mg):
                    pss = [pups.tile([tok_block, 512], F32, tag="ps",
                                     name=f"ps_{n}_{mg}_{i}")
                           for i in range(mg_w)]
                    for k in range(16):
                        wt = puw.tile([128, 512], BF16, tag="wt")
                        nc.sync.dma_start(wt[:], wih_d_d[k, :, n * 512:(n + 1) * 512])
                        src = fwd_hT if k < 8 else bwd_hT
                        for m in range(mg_w):
                            mb = mg * mg_w + m
                            nc.tensor.matmul(
                                pss[m][:],
                                src[:, k % 8, mb * tok_block:(mb + 1) * tok_block],
                                wt[:], start=(k == 0), stop=(k == 15))
                    for m in range(mg_w):
                        mb = mg * mg_w + m
                        ev = puev.tile([tok_block, 512], F32, tag="ev")
                        nc.vector.tensor_add(
                            ev[:], pss[m][:],
                            bias_sb[:tok_block, n * 512:(n + 1) * 512])
                        nc.sync.dma_start(
                            ug_d[mb * tok_block:(mb + 1) * tok_block,
                                 n * 512:(n + 1) * 512], ev[:])

        # ---------------- phase R2: decoder recurrence -----------------------
        dec_hT = hist.tile([128, 8, TOK], BF16, tag="hist")
        recurrence("rd", whh_d_d, ug_d, dec_hT, reverse=False, dec_first=decb0_d)

        # ---------------- phase P: vocab projection --------------------------
        with (
            tc.tile_pool(name="pp", bufs=1) as pp,
            tc.tile_pool(name="pp_ps", bufs=2, space="PSUM") as ppps,
            tc.tile_pool(name="pp_ev", bufs=2) as ppev,
        ):
            wout_sb = pp.tile([128, 8, V], BF16, tag="wout")
            for k in range(8):
                nc.sync.dma_start(wout_sb[:, k, :], wout_d[k])
            outb_sb = pp.tile([V, 1], F32, tag="outb")
            nc.sync.dma_start(outb_sb[:], outb_d[:])
            for n in range(n_tok_chunks):
                ps = ppps.tile([V, tok_chunk], F32, tag="ps")
                for k in range(8):
                    nc.tensor.matmul(
                        ps[:], wout_sb[:, k, :],
                        dec_hT[:, k, n * tok_chunk:(n + 1) * tok_chunk],
                        start=(k == 0), stop=(k == 7))
                ev = ppev.tile([V, tok_chunk], F32, tag="ev")
                nc.vector.tensor_scalar_add(ev[:], ps[:], outb_sb[:])
                nc.sync.dma_start(pred_d[:, n * tok_chunk:(n + 1) * tok_chunk], ev[:])

    return nc


_CACHE = {}


def _get_nc(n_steps):
    if n_steps not in _CACHE:
        nc = bacc.Bacc("TRN2", target_bir_lowering=False, debug=False)
        _build(nc, n_steps)
        nc.compile()
        _CACHE[n_steps] = nc
    return _CACHE[n_steps]


def _gate_perm():
    r = np.arange(G4)
    return np.concatenate([r[0:H], r[2 * H:3 * H], r[H:2 * H], r[3 * H:4 * H]])


def _prep_shared(embedding, enc_w_ih_f, enc_w_hh_f, enc_b_f, enc_w_ih_b,
                 enc_w_hh_b, enc_b_b, dec_w_ih, dec_w_hh, dec_b, out_w, out_b):
    p = _gate_perm()

    def wT(w, kt):
        return np.ascontiguousarray(
            w[p].T.reshape(kt, 128, G4).astype(bf16_np))

    embp = np.zeros((VP, E), np.float32)
    embp[:V] = embedding
    shared = {
        "embp": embp.astype(bf16_np),
        "wih_f": wT(enc_w_ih_f, 4),
        "wih_b": wT(enc_w_ih_b, 4),
        "whh_f": wT(enc_w_hh_f, 8),
        "whh_b": wT(enc_w_hh_b, 8),
        "wih_d": wT(dec_w_ih, 16),
        "whh_d": wT(dec_w_hh, 8),
        "bias_f": np.broadcast_to(enc_b_f[p], (128, G4)).astype(np.float32).copy(),
        "bias_b": np.broadcast_to(enc_b_b[p], (128, G4)).astype(np.float32).copy(),
        "bias_d": np.broadcast_to(dec_b[p], (128, G4)).astype(np.float32).copy(),
        "decb0": np.broadcast_to(dec_b[p], (BL, G4)).astype(np.float32).copy(),
        "wout": np.ascontiguousarray(
            out_w.T.reshape(8, 128, V).astype(bf16_np)),
        "outb": out_b.reshape(V, 1).astype(np.float32),
        "ident": np.eye(BL, dtype=bf16_np),
    }
    return shared


def _in_maps(inputs, n_steps):
    input_seq = np.asarray(inputs["input_seq"]).astype(np.int64)
    shared = _prep_shared(
        *[np.asarray(inputs[k], np.float32) for k in (
            "embedding", "enc_w_ih_f", "enc_w_hh_f", "enc_b_f",
            "enc_w_ih_b", "enc_w_hh_b", "enc_b_b",
            "dec_w_ih", "dec_w_hh", "dec_b", "out_w", "out_b")])
    TOK = n_steps * BL
    in_maps = []
    for c in range(NCORES):
        idx = input_seq[:n_steps, c * BL:(c + 1) * BL]  # [n_steps, BL]
        oh = np.zeros((VP, TOK), np.float32)
        cols = np.arange(TOK)
        oh[idx.reshape(-1), cols] = 1.0
        m = dict(shared)
        m["onehot"] = oh.astype(bf16_np)
        in_maps.append(m)
    return in_maps


def _assemble(res, n_steps):
    outs = []
    for c in range(NCORES):
        pr = res.results[c]["pred"]            # [V, TOK]
        outs.append(pr.reshape(V, n_steps, BL).transpose(1, 2, 0))
    return np.concatenate(outs, axis=1).astype(np.float32)  # [n_steps, B, V]


def _run(inputs, n_steps):
    in_maps = _in_maps(inputs, n_steps)
    nc = _get_nc(n_steps)
    res = run_bass_kernel_spmd(nc, in_maps, core_ids=list(range(NCORES)))
    return _assemble(res, n_steps)


def _register_ntff_hook():
    """Make antenv.axon_hooks importable (the image's antenv lacks it)."""
    import importlib.util
    if "antenv.axon_hooks" in sys.modules:
        return
    path = "/opt/trn_rl_repo/antenv/axon_hooks.py"
    if not os.path.exists(path):
        return
    spec = importlib.util.spec_from_file_location("antenv.axon_hooks", path)
    mod = importlib.util.module_from_spec(spec)
    spec.loader.exec_module(mod)
    sys.modules["antenv.axon_hooks"] = mod


def _run_traced(inputs, n_steps):
    _register_ntff_hook()
    in_maps = _in_maps(inputs, n_steps)
    nc = _get_nc(n_steps)
    res = run_bass_kernel_spmd(nc, in_maps, core_ids=list(range(NCORES)),
                               trace=True)
    return _assemble(res, n_steps), res


def kernel(**inputs):
    return _run(inputs, S)


# revision 13
# speedup vs baseline: 10.0631x; 1.2034x over previous
"""Trainium2 Bass kernel for nn_CharStemmer (bi-LSTM encoder + LSTM decoder).

Sharding: data-parallel over batch (B=128) across 8 cores, 16 sequences per
core; all weights replicated. Inside each core:
  - embedding lookup as one-hot matmul
  - input-side gate GEMMs (xg = emb @ w_ih^T + b) batched over all timesteps
  - the three recurrences run step-by-step; per step the stationary matmul
    operand is h^T (tiny) and w_hh^T streams through the PE in bf16
  - h is computed batch-major [16, 1024] for full-width elementwise, then
    PE-transposed into the hidden-major history h^T used as next-step lhsT
  - final vocab projection from the stored h^T history.
"""

import os
import sys

for _p in ("/opt/trn_rl_repo", "/root/.axon_site/_ro/trn_rl_repo"):
    if os.path.isdir(_p) and _p not in sys.path:
        sys.path.insert(0, _p)

from contextlib import ExitStack

import ml_dtypes
import numpy as np

import concourse.bass as bass
import concourse.tile as tile
from concourse import bacc, mybir
from concourse.bass_utils import run_bass_kernel_spmd

S, B, V, E, H = 128, 128, 61, 512, 1024
NCORES = 8
BL = B // NCORES          # 16 sequences per core
G4 = 4 * H                # 4096 gate columns
VP = 64                   # vocab padded to 64 partitions
BF16 = mybir.dt.bfloat16
F32 = mybir.dt.float32
AF = mybir.ActivationFunctionType
bf16_np = ml_dtypes.bfloat16

# gate quarters in permuted order: q0=i, q1=g, q2=f, q3=o


def _build(nc, n_steps):
    TOK = n_steps * BL

    def din(name, shape, dt):
        return nc.dram_tensor(name, list(shape), dt, kind="ExternalInput").ap()

    onehot_d = din("onehot", [VP, TOK], BF16)
    embp_d = din("embp", [VP, E], BF16)
    wih_f_d = din("wih_f", [4, 128, G4], BF16)
    wih_b_d = din("wih_b", [4, 128, G4], BF16)
    whh_f_d = din("whh_f", [8, 128, G4], BF16)
    whh_b_d = din("whh_b", [8, 128, G4], BF16)
    wih_d_d = din("wih_d", [16, 128, G4], BF16)
    whh_d_d = din("whh_d", [8, 128, G4], BF16)
    bias_f_d = din("bias_f", [128, G4], F32)
    bias_b_d = din("bias_b", [128, G4], F32)
    bias_d_d = din("bias_d", [128, G4], F32)
    decb0_d = din("decb0", [BL, G4], BF16)   # decoder step-0 gates (bias only)
    wout_d = din("wout", [8, 128, V], BF16)
    outb_d = din("outb", [V, 1], F32)
    ident_d = din("ident", [BL, BL], BF16)
    pred_d = nc.dram_tensor("pred", [V, TOK], F32, kind="ExternalOutput").ap()

    with ExitStack() as ctx:
        tc = ctx.enter_context(tile.TileContext(nc))
        dram = ctx.enter_context(tc.tile_pool(name="dram", bufs=1, space="DRAM"))
        xg_f = dram.tile([TOK, G4], BF16, tag="xgf")
        xg_b = dram.tile([TOK, G4], BF16, tag="xgb")
        ug_d = dram.tile([TOK, G4], BF16, tag="ugd")

        persist = ctx.enter_context(tc.tile_pool(name="persist", bufs=1))
        hist = ctx.enter_context(tc.tile_pool(name="hist", bufs=2))

        ident_sb = persist.tile([BL, BL], BF16, tag="ident")
        nc.sync.dma_start(ident_sb[:], ident_d[:])

        # ---------------- phase E+X: embedding + input-side gate GEMMs -------
        n_tok_chunks = TOK // 512 if TOK >= 512 else 1
        tok_chunk = min(TOK, 512)
        n_tok_blocks = TOK // 128 if TOK >= 128 else 1
        tok_block = min(TOK, 128)

        with (
            tc.tile_pool(name="px", bufs=1) as px,
            tc.tile_pool(name="px_ps", bufs=8, space="PSUM") as px_ps,
            tc.tile_pool(name="px_ev", bufs=4) as px_ev,
        ):
            oh_sb = px.tile([VP, TOK], BF16, tag="oh")
            for j in range(TOK // 2048 if TOK >= 2048 else 1):
                w = min(TOK, 2048)
                nc.sync.dma_start(oh_sb[:, j * w:(j + 1) * w],
                                  onehot_d[:, j * w:(j + 1) * w])
            embp_sb = px.tile([VP, E], BF16, tag="embp")
            nc.sync.dma_start(embp_sb[:], embp_d[:])
            embT_sb = px.tile([128, 4 * TOK], BF16, tag="embT")

            # emb^T[e_chunk, tok] = embp.T @ onehot
            for m in range(4):
                for n in range(n_tok_chunks):
                    ps = px_ps.tile([128, tok_chunk], F32, tag="ps")
                    nc.tensor.matmul(
                        ps[:], embp_sb[:, m * 128:(m + 1) * 128],
                        oh_sb[:, n * tok_chunk:(n + 1) * tok_chunk],
                        start=True, stop=True)
                    nc.vector.tensor_copy(
                        embT_sb[:, m * TOK + n * tok_chunk:
                                m * TOK + (n + 1) * tok_chunk], ps[:])

            # xg = emb @ w_ih^T + b   (token-major [TOK, G4], f32, to DRAM)
            for wih_src, bias_src, xg_dst in (
                (wih_f_d, bias_f_d, xg_f),
                (wih_b_d, bias_b_d, xg_b),
            ):
                wih_sb = px.tile([128, 4, G4], BF16, tag="wih")
                for k in range(4):
                    nc.sync.dma_start(wih_sb[:, k, :], wih_src[k])
                bias_sb = px.tile([128, G4], F32, tag="bias")
                nc.sync.dma_start(bias_sb[:], bias_src[:])
                for m in range(n_tok_blocks):
                    for n in range(8):
                        ps = px_ps.tile([tok_block, 512], F32, tag="ps")
                        for k in range(4):
                            nc.tensor.matmul(
                                ps[:],
                                embT_sb[:, k * TOK + m * tok_block:
                                        k * TOK + (m + 1) * tok_block],
                                wih_sb[:, k, n * 512:(n + 1) * 512],
                                start=(k == 0), stop=(k == 3))
                        ev = px_ev.tile([tok_block, 512], BF16, tag="ev")
                        nc.vector.tensor_add(
                            ev[:], ps[:], bias_sb[:tok_block, n * 512:(n + 1) * 512])
                        nc.sync.dma_start(
                            xg_dst[m * tok_block:(m + 1) * tok_block,
                                   n * 512:(n + 1) * 512], ev[:])

        # ---------------- recurrence helper ---------------------------------
        # quarter order in xg cols is (i,g,f,o); col-group map: i->0 f->1 o->2 g->3
        QGRP = [0, 3, 1, 2]

        def recurrence(name, whh_src, xg_src, hT, reverse, dec_first=None):
            """One LSTM over n_steps. hT: [128, 8, TOK] bf16 history tile.
            Gates land col-tiled in one PSUM tile [128,1024]: quarter q at
            partitions [32*QGRP[q], +16). xg is accumulated into PSUM via an
            identity matmul. c lives in PSUM so DVE ops may mix bases."""
            whh_sb = persist.tile([128, 8, G4], BF16, tag="whh")
            for k in range(8):
                nc.sync.dma_start(whh_sb[:, k, :], whh_src[k])

            with (
                tc.tile_pool(name=name + "_ew", bufs=1) as ew,
                tc.tile_pool(name=name + "_xq", bufs=3) as xqp,
                tc.tile_pool(name=name + "_q", bufs=2, space="PSUM") as qps,
                tc.tile_pool(name=name + "_c", bufs=1, space="PSUM") as cpsp,
                tc.tile_pool(name=name + "_tr", bufs=2, space="PSUM") as trps,
            ):
                c_ps = cpsp.tile([BL, H], F32, tag="c")

                def tr_half(h, nn, t):
                    trb = trps.tile([128, 4 * BL], BF16, tag="tr",
                                    name=f"trb_{name}_{t}_{nn}")
                    for j in range(4):
                        kk = nn * 4 + j
                        nc.tensor.transpose(
                            trb[:, j * BL:(j + 1) * BL],
                            h[:, kk * 128:(kk + 1) * 128], ident_sb[:])
                    trb_r = trb.rearrange("p (k b) -> p k b", k=4)
                    nc.vector.tensor_copy(
                        hT[:, nn * 4:(nn + 1) * 4, t * BL:(t + 1) * BL],
                        trb_r[:, :, :])

                for s in range(n_steps):
                    t = (n_steps - 1 - s) if reverse else s
                    tprev = (n_steps - s) if reverse else (s - 1)
                    xq = xqp.tile([BL, G4], BF16, tag="xq")
                    if dec_first is not None and t == 0:
                        nc.sync.dma_start(xq[:], dec_first[:])
                    else:
                        tsrc = (t - 1) if dec_first is not None else t
                        nc.sync.dma_start(
                            xq[:], xg_src[tsrc * BL:(tsrc + 1) * BL, :])

                    if s == 0:
                        # gates = xq only (h=c=0); quarters are xq col blocks
                        si0 = ew.tile([BL, H], F32, tag="si0")
                        nc.scalar.activation(si0[:], xq[:, 0:H], AF.Sigmoid)
                        tg0 = ew.tile([BL, H], F32, tag="tg0")
                        nc.scalar.activation(tg0[:], xq[:, H:2 * H], AF.Tanh)
                        a0 = ew.tile([BL, H], F32, tag="a0")
                        nc.vector.tensor_mul(a0[:], si0[:], tg0[:])
                        nc.vector.tensor_copy(c_ps[:], a0[:])
                        tc0 = ew.tile([BL, H], F32, tag="tc0")
                        nc.scalar.activation(tc0[:], c_ps[:], AF.Tanh)
                        so0 = ew.tile([BL, H], F32, tag="so0")
                        nc.scalar.activation(so0[:], xq[:, 3 * H:4 * H],
                                             AF.Sigmoid)
                        h = ew.tile([BL, H], BF16, tag="h", bufs=2)
                        nc.vector.tensor_mul(h[:], so0[:], tc0[:])
                        for nn in range(2):
                            tr_half(h, nn, t)
                        continue

                    ps = qps.tile([128, H], F32, tag="q")
                    for nn in range(2):
                        sl = slice(nn * 512, (nn + 1) * 512)
                        for q in range(4):
                            g = QGRP[q]
                            nc.tensor.matmul(
                                ps[32 * g:32 * g + 16, sl], ident_sb[:],
                                xq[:, q * H + nn * 512:q * H + (nn + 1) * 512],
                                start=True, stop=False,
                                tile_position=(0, 32 * g))
                        for k in range(8):
                            lhsT = hT[:, k, tprev * BL:(tprev + 1) * BL]
                            for q in range(4):
                                g = QGRP[q]
                                nc.tensor.matmul(
                                    ps[32 * g:32 * g + 16, sl], lhsT,
                                    whh_sb[:, k, q * H + nn * 512:
                                           q * H + (nn + 1) * 512],
                                    start=False, stop=(k == 7),
                                    tile_position=(0, 32 * g))

                    h = ew.tile([BL, H], BF16, tag="h", bufs=2)
                    for nn in range(2):
                        sl = slice(nn * 512, (nn + 1) * 512)
                        s_if = ew.tile([48, 512], F32, tag="sif", bufs=2)
                        nc.scalar.activation(s_if[:], ps[0:48, sl], AF.Sigmoid)
                        so = ew.tile([BL, 512], F32, tag="so", bufs=2)
                        nc.scalar.activation(so[:], ps[64:80, sl], AF.Sigmoid)
                        tg = ew.tile([BL, 512], F32, tag="tg", bufs=2)
                        nc.scalar.activation(tg[:], ps[96:112, sl], AF.Tanh)
                        a = ew.tile([BL, 512], F32, tag="a", bufs=2)
                        nc.vector.tensor_mul(a[:], s_if[0:BL, :], tg[:])
                        t1 = ew.tile([BL, 512], F32, tag="t1", bufs=2)
                        nc.vector.tensor_mul(t1[:], s_if[32:48, :],
                                             c_ps[:, sl])
                        nc.vector.tensor_add(c_ps[:, sl], t1[:], a[:])
                        tc_ = ew.tile([BL, 512], F32, tag="tc", bufs=2)
                        nc.scalar.activation(tc_[:], c_ps[:, sl], AF.Tanh)
                        nc.vector.tensor_mul(h[:, sl], so[:], tc_[:])
                        tr_half(h, nn, t)

        # ---------------- phase R1: encoder fwd + bwd ------------------------
        fwd_hT = hist.tile([128, 8, TOK], BF16, tag="hist")
        recurrence("rf", whh_f_d, xg_f, fwd_hT, reverse=False)
        bwd_hT = hist.tile([128, 8, TOK], BF16, tag="hist")
        recurrence("rb", whh_b_d, xg_b, bwd_hT, reverse=True)

        # ---------------- phase U: decoder input GEMM ------------------------
        # ug[tok] = encoded[tok] @ dec_w_ih^T + dec_b   (unshifted; read at t-1)
        with (
            tc.tile_pool(name="pu", bufs=1) as pu,
            tc.tile_pool(name="pu_w", bufs=3) as puw,
            tc.tile_pool(name="pu_ps", bufs=8, space="PSUM") as pups,
            tc.tile_pool(name="pu_ev", bufs=4) as puev,
        ):
            bias_sb = pu.tile([128, G4], F32, tag="biasd")
            nc.sync.dma_start(bias_sb[:], bias_d_d[:])
            n_mg = max(1, n_tok_blocks // 8)
            mg_w = min(8, n_tok_blocks)
            for n in range(8):
                for mg in range(n_mg):
                    pss = [pups.tile([tok_block, 512], F32, tag="ps",
                                     name=f"ps_{n}_{mg}_{i}")
                           for i in range(mg_w)]
                    for k in range(16):
                        wt = puw.tile([128, 512], BF16, tag="wt")
                        nc.sync.dma_start(wt[:], wih_d_d[k, :, n * 512:(n + 1) * 512])
                        src = fwd_hT if k < 8 else bwd_hT
                        for m in range(mg_w):
                            mb = mg * mg_w + m
                            nc.tensor.matmul(
                                pss[m][:],
                                src[:, k % 8, mb * tok_block:(mb + 1) * tok_block],
                                wt[:], start=(k == 0), stop=(k == 15))
                    for m in range(mg_w):
                        mb = mg * mg_w + m
                        ev = puev.tile([tok_block, 512], BF16, tag="ev")
                        nc.vector.tensor_add(
                            ev[:], pss[m][:],
                            bias_sb[:tok_block, n * 512:(n + 1) * 512])
                        nc.sync.dma_start(
                            ug_d[mb * tok_block:(mb + 1) * tok_block,
                                 n * 512:(n + 1) * 512], ev[:])

        # ---------------- phase R2: decoder recurrence -----------------------
        dec_hT = hist.tile([128, 8, TOK], BF16, tag="hist")
        recurrence("rd", whh_d_d, ug_d, dec_hT, reverse=False, dec_first=decb0_d)

        # ---------------- phase P: vocab projection --------------------------
        with (
            tc.tile_pool(name="pp", bufs=1) as pp,
            tc.tile_pool(name="pp_ps", bufs=2, space="PSUM") as ppps,
            tc.tile_pool(name="pp_ev", bufs=2) as ppev,
        ):
            wout_sb = pp.tile([128, 8, V], BF16, tag="wout")
            for k in range(8):
                nc.sync.dma_start(wout_sb[:, k, :], wout_d[k])
            outb_sb = pp.tile([V, 1], F32, tag="outb")
            nc.sync.dma_start(outb_sb[:], outb_d[:])
            for n in range(n_tok_chunks):
                ps = ppps.tile([V, tok_chunk], F32, tag="ps")
                for k in range(8):
                    nc.tensor.matmul(
                        ps[:], wout_sb[:, k, :],
                        dec_hT[:, k, n * tok_chunk:(n + 1) * tok_chunk],
                        start=(k == 0), stop=(k == 7))
                ev = ppev.tile([V, tok_chunk], F32, tag="ev")
                nc.vector.tensor_scalar_add(ev[:], ps[:], outb_sb[:])
                nc.sync.dma_start(pred_d[:, n * tok_chunk:(n + 1) * tok_chunk], ev[:])

    return nc


_CACHE = {}


def _get_nc(n_steps):
    if n_steps not in _CACHE:
        nc = bacc.Bacc("TRN2", target_bir_lowering=False, debug=False)
        _build(nc, n_steps)
        nc.compile()
        _CACHE[n_steps] = nc
    return _CACHE[n_steps]


def _gate_perm():
    r = np.arange(G4)
    return np.concatenate([r[0:H], r[2 * H:3 * H], r[H:2 * H], r[3 * H:4 * H]])


def _prep_shared(embedding, enc_w_ih_f, enc_w_hh_f, enc_b_f, enc_w_ih_b,
                 enc_w_hh_b, enc_b_b, dec_w_ih, dec_w_hh, dec_b, out_w, out_b):
    p = _gate_perm()

    def wT(w, kt):
        return np.ascontiguousarray(
            w[p].T.reshape(kt, 128, G4).astype(bf16_np))

    embp = np.zeros((VP, E), np.float32)
    embp[:V] = embedding
    shared = {
        "embp": embp.astype(bf16_np),
        "wih_f": wT(enc_w_ih_f, 4),
        "wih_b": wT(enc_w_ih_b, 4),
        "whh_f": wT(enc_w_hh_f, 8),
        "whh_b": wT(enc_w_hh_b, 8),
        "wih_d": wT(dec_w_ih, 16),
        "whh_d": wT(dec_w_hh, 8),
        "bias_f": np.broadcast_to(enc_b_f[p], (128, G4)).astype(np.float32).copy(),
        "bias_b": np.broadcast_to(enc_b_b[p], (128, G4)).astype(np.float32).copy(),
        "bias_d": np.broadcast_to(dec_b[p], (128, G4)).astype(np.float32).copy(),
        "decb0": np.broadcast_to(dec_b[p], (BL, G4)).astype(bf16_np).copy(),
        "wout": np.ascontiguousarray(
            out_w.T.reshape(8, 128, V).astype(bf16_np)),
        "outb": out_b.reshape(V, 1).astype(np.float32),
        "ident": np.eye(BL, dtype=bf16_np),
    }
    return shared


def _in_maps(inputs, n_steps):
    input_seq = np.asarray(inputs["input_seq"]).astype(np.int64)
    shared = _prep_shared(
        *[np.asarray(inputs[k], np.float32) for k in (
            "embedding", "enc_w_ih_f", "enc_w_hh_f", "enc_b_f",
            "enc_w_ih_b", "enc_w_hh_b", "enc_b_b",
            "dec_w_ih", "dec_w_hh", "dec_b", "out_w", "out_b")])
    TOK = n_steps * BL
    in_maps = []
    for c in range(NCORES):
        idx = input_seq[:n_steps, c * BL:(c + 1) * BL]  # [n_steps, BL]
        oh = np.zeros((VP, TOK), np.float32)
        cols = np.arange(TOK)
        oh[idx.reshape(-1), cols] = 1.0
        m = dict(shared)
        m["onehot"] = oh.astype(bf16_np)
        in_maps.append(m)
    return in_maps


def _assemble(res, n_steps):
    outs = []
    for c in range(NCORES):
        pr = res.results[c]["pred"]            # [V, TOK]
        outs.append(pr.reshape(V, n_steps, BL).transpose(1, 2, 0))
    return np.concatenate(outs, axis=1).astype(np.float32)  # [n_steps, B, V]


def _run(inputs, n_steps):
    in_maps = _in_maps(inputs, n_steps)
    nc = _get_nc(n_steps)
    res = run_bass_kernel_spmd(nc, in_maps, core_ids=list(range(NCORES)))
    return _assemble(res, n_steps)


def _register_ntff_hook():
    """Make antenv.axon_hooks importable (the image's antenv lacks it)."""
    import importlib.util
    if "antenv.axon_hooks" in sys.modules:
        return
    path = "/opt/trn_rl_repo/antenv/axon_hooks.py"
    if not os.path.exists(path):
        return
    spec = importlib.util.spec_from_file_location("antenv.axon_hooks", path)
    mod = importlib.util.module_from_spec(spec)
    spec.loader.exec_module(mod)
    sys.modules["antenv.axon_hooks"] = mod


def _run_traced(inputs, n_steps):
    _register_ntff_hook()
    in_maps = _in_maps(inputs, n_steps)
    nc = _get_nc(n_steps)
    res = run_bass_kernel_spmd(nc, in_maps, core_ids=list(range(NCORES)),
                               trace=True)
    return _assemble(res, n_steps), res


def kernel(**inputs):
    return _run(inputs, S)


# revision 14
# speedup vs baseline: 12.6532x; 1.2574x over previous
"""Trainium2 Bass kernel for nn_CharStemmer (bi-LSTM encoder + LSTM decoder).

Sharding: data-parallel over batch (B=128) across 8 cores, 16 sequences per
core; all weights replicated. Inside each core:
  - embedding lookup as one-hot matmul
  - input-side gate GEMMs (xg = emb @ w_ih^T + b) batched over all timesteps
  - the three recurrences run step-by-step; per step the stationary matmul
    operand is h^T (tiny) and w_hh^T streams through the PE in bf16
  - h is computed batch-major [16, 1024] for full-width elementwise, then
    PE-transposed into the hidden-major history h^T used as next-step lhsT
  - final vocab projection from the stored h^T history.
"""

import os
import sys

for _p in ("/opt/trn_rl_repo", "/root/.axon_site/_ro/trn_rl_repo"):
    if os.path.isdir(_p) and _p not in sys.path:
        sys.path.insert(0, _p)

from contextlib import ExitStack

import ml_dtypes
import numpy as np

import concourse.bass as bass
import concourse.tile as tile
from concourse import bacc, mybir
from concourse.bass_utils import run_bass_kernel_spmd

S, B, V, E, H = 128, 128, 61, 512, 1024
NCORES = 8
BL = B // NCORES          # 16 sequences per core
G4 = 4 * H                # 4096 gate columns
VP = 64                   # vocab padded to 64 partitions
BF16 = mybir.dt.bfloat16
F32 = mybir.dt.float32
AF = mybir.ActivationFunctionType
bf16_np = ml_dtypes.bfloat16

# gate quarters in permuted order: q0=i, q1=g, q2=f, q3=o


def _build(nc, n_steps):
    TOK = n_steps * BL

    def din(name, shape, dt):
        return nc.dram_tensor(name, list(shape), dt, kind="ExternalInput").ap()

    onehot_d = din("onehot", [VP, TOK], BF16)
    embp_d = din("embp", [VP, E], BF16)
    wih_f_d = din("wih_f", [4, 128, G4], BF16)
    wih_b_d = din("wih_b", [4, 128, G4], BF16)
    whh_f_d = din("whh_f", [8, 128, G4], BF16)
    whh_b_d = din("whh_b", [8, 128, G4], BF16)
    wih_d_d = din("wih_d", [16, 128, G4], BF16)
    whh_d_d = din("whh_d", [8, 128, G4], BF16)
    bias_f_d = din("bias_f", [128, G4], F32)
    bias_b_d = din("bias_b", [128, G4], F32)
    bias_d_d = din("bias_d", [128, G4], F32)
    decb0_d = din("decb0", [BL, G4], BF16)   # decoder step-0 gates (bias only)
    wout_d = din("wout", [8, 128, V], BF16)
    outb_d = din("outb", [V, 1], F32)
    ident_d = din("ident", [BL, BL], BF16)
    pred_d = nc.dram_tensor("pred", [V, TOK], F32, kind="ExternalOutput").ap()

    with ExitStack() as ctx:
        tc = ctx.enter_context(tile.TileContext(nc))
        dram = ctx.enter_context(tc.tile_pool(name="dram", bufs=1, space="DRAM"))
        xg_f = dram.tile([TOK, G4], BF16, tag="xgf")
        xg_b = dram.tile([TOK, G4], BF16, tag="xgb")
        ug_d = dram.tile([TOK, G4], BF16, tag="ugd")

        persist = ctx.enter_context(tc.tile_pool(name="persist", bufs=1))
        hist = ctx.enter_context(tc.tile_pool(name="hist", bufs=2))

        ident_sb = persist.tile([BL, BL], BF16, tag="ident")
        nc.sync.dma_start(ident_sb[:], ident_d[:])

        # ---------------- phase E+X: embedding + input-side gate GEMMs -------
        n_tok_chunks = TOK // 512 if TOK >= 512 else 1
        tok_chunk = min(TOK, 512)
        n_tok_blocks = TOK // 128 if TOK >= 128 else 1
        tok_block = min(TOK, 128)

        with (
            tc.tile_pool(name="px", bufs=1) as px,
            tc.tile_pool(name="px_ps", bufs=8, space="PSUM") as px_ps,
            tc.tile_pool(name="px_ev", bufs=4) as px_ev,
        ):
            oh_sb = px.tile([VP, TOK], BF16, tag="oh")
            for j in range(TOK // 2048 if TOK >= 2048 else 1):
                w = min(TOK, 2048)
                nc.sync.dma_start(oh_sb[:, j * w:(j + 1) * w],
                                  onehot_d[:, j * w:(j + 1) * w])
            embp_sb = px.tile([VP, E], BF16, tag="embp")
            nc.sync.dma_start(embp_sb[:], embp_d[:])
            embT_sb = px.tile([128, 4 * TOK], BF16, tag="embT")

            # emb^T[e_chunk, tok] = embp.T @ onehot
            for m in range(4):
                for n in range(n_tok_chunks):
                    ps = px_ps.tile([128, tok_chunk], F32, tag="ps")
                    nc.tensor.matmul(
                        ps[:], embp_sb[:, m * 128:(m + 1) * 128],
                        oh_sb[:, n * tok_chunk:(n + 1) * tok_chunk],
                        start=True, stop=True)
                    nc.vector.tensor_copy(
                        embT_sb[:, m * TOK + n * tok_chunk:
                                m * TOK + (n + 1) * tok_chunk], ps[:])

            # xg = emb @ w_ih^T + b   (token-major [TOK, G4], f32, to DRAM)
            for wih_src, bias_src, xg_dst in (
                (wih_f_d, bias_f_d, xg_f),
                (wih_b_d, bias_b_d, xg_b),
            ):
                wih_sb = px.tile([128, 4, G4], BF16, tag="wih")
                for k in range(4):
                    nc.sync.dma_start(wih_sb[:, k, :], wih_src[k])
                bias_sb = px.tile([128, G4], F32, tag="bias")
                nc.sync.dma_start(bias_sb[:], bias_src[:])
                for m in range(n_tok_blocks):
                    for n in range(8):
                        ps = px_ps.tile([tok_block, 512], F32, tag="ps")
                        for k in range(4):
                            nc.tensor.matmul(
                                ps[:],
                                embT_sb[:, k * TOK + m * tok_block:
                                        k * TOK + (m + 1) * tok_block],
                                wih_sb[:, k, n * 512:(n + 1) * 512],
                                start=(k == 0), stop=(k == 3))
                        ev = px_ev.tile([tok_block, 512], BF16, tag="ev")
                        nc.vector.tensor_add(
                            ev[:], ps[:], bias_sb[:tok_block, n * 512:(n + 1) * 512])
                        nc.sync.dma_start(
                            xg_dst[m * tok_block:(m + 1) * tok_block,
                                   n * 512:(n + 1) * 512], ev[:])

        # ---------------- recurrence helper ---------------------------------
        # quarter order in xg cols is (i,g,f,o); col-group map: i->0 f->1 o->2 g->3
        QGRP = [0, 3, 1, 2]

        def recurrence(name, whh_src, xg_src, hT, reverse, dec_first=None):
            """One LSTM over n_steps. hT: [128, 8, TOK] bf16 history tile.
            Gates land col-tiled in one PSUM tile [128,1024]: quarter q at
            partitions [32*QGRP[q], +16). xg is accumulated into PSUM via an
            identity matmul. c lives in PSUM so DVE ops may mix bases."""
            whh_sb = persist.tile([128, 8, G4], BF16, tag="whh")
            for k in range(8):
                nc.sync.dma_start(whh_sb[:, k, :], whh_src[k])

            with (
                tc.tile_pool(name=name + "_ew", bufs=1) as ew,
                tc.tile_pool(name=name + "_xq", bufs=3) as xqp,
                tc.tile_pool(name=name + "_q", bufs=2, space="PSUM") as qps,
                tc.tile_pool(name=name + "_c", bufs=1, space="PSUM") as cpsp,
                tc.tile_pool(name=name + "_tr", bufs=2, space="PSUM") as trps,
            ):
                c_ps = cpsp.tile([BL, H], F32, tag="c")

                def tr_half(h, nn, t):
                    trb = trps.tile([128, 4 * BL], BF16, tag="tr",
                                    name=f"trb_{name}_{t}_{nn}")
                    for j in range(4):
                        kk = nn * 4 + j
                        nc.tensor.transpose(
                            trb[:, j * BL:(j + 1) * BL],
                            h[:, kk * 128:(kk + 1) * 128], ident_sb[:])
                    trb_r = trb.rearrange("p (k b) -> p k b", k=4)
                    nc.vector.tensor_copy(
                        hT[:, nn * 4:(nn + 1) * 4, t * BL:(t + 1) * BL],
                        trb_r[:, :, :])

                for s in range(n_steps):
                    t = (n_steps - 1 - s) if reverse else s
                    tprev = (n_steps - s) if reverse else (s - 1)
                    xq = xqp.tile([BL, G4], BF16, tag="xq")
                    if dec_first is not None and t == 0:
                        nc.sync.dma_start(xq[:], dec_first[:])
                    else:
                        tsrc = (t - 1) if dec_first is not None else t
                        nc.sync.dma_start(
                            xq[:], xg_src[tsrc * BL:(tsrc + 1) * BL, :])

                    if s == 0:
                        # gates = xq only (h=c=0); quarters are xq col blocks
                        si0 = ew.tile([BL, H], F32, tag="si0")
                        nc.scalar.activation(si0[:], xq[:, 0:H], AF.Sigmoid)
                        tg0 = ew.tile([BL, H], F32, tag="tg0")
                        nc.scalar.activation(tg0[:], xq[:, H:2 * H], AF.Tanh)
                        a0 = ew.tile([BL, H], F32, tag="a0")
                        nc.vector.tensor_mul(a0[:], si0[:], tg0[:])
                        nc.vector.tensor_copy(c_ps[:], a0[:])
                        tc0 = ew.tile([BL, H], F32, tag="tc0")
                        nc.scalar.activation(tc0[:], c_ps[:], AF.Tanh)
                        so0 = ew.tile([BL, H], F32, tag="so0")
                        nc.scalar.activation(so0[:], xq[:, 3 * H:4 * H],
                                             AF.Sigmoid)
                        h = ew.tile([BL, H], BF16, tag="h", bufs=2)
                        nc.vector.tensor_mul(h[:], so0[:], tc0[:])
                        for nn in range(2):
                            tr_half(h, nn, t)
                        continue

                    ps = qps.tile([128, H], F32, tag="q")
                    for nn in range(2):
                        sl = slice(nn * 512, (nn + 1) * 512)
                        for q in range(4):
                            g = QGRP[q]
                            nc.tensor.matmul(
                                ps[32 * g:32 * g + 16, sl], ident_sb[:],
                                xq[:, q * H + nn * 512:q * H + (nn + 1) * 512],
                                start=True, stop=False,
                                tile_position=(0, 32 * g))
                        for k in range(8):
                            lhsT = hT[:, k, tprev * BL:(tprev + 1) * BL]
                            for q in range(4):
                                g = QGRP[q]
                                nc.tensor.matmul(
                                    ps[32 * g:32 * g + 16, sl], lhsT,
                                    whh_sb[:, k, q * H + nn * 512:
                                           q * H + (nn + 1) * 512],
                                    start=False, stop=(k == 7),
                                    tile_position=(0, 32 * g))

                    h = ew.tile([BL, H], BF16, tag="h", bufs=2)
                    for nn in range(2):
                        sl = slice(nn * 512, (nn + 1) * 512)
                        tg = ew.tile([BL, 512], F32, tag="tg", bufs=2)
                        nc.scalar.activation(tg[:], ps[96:112, sl], AF.Tanh)
                        sio = ew.tile([80, 512], F32, tag="sio", bufs=2)
                        nc.scalar.activation(sio[:], ps[0:80, sl], AF.Sigmoid)
                        t1 = ew.tile([BL, 512], F32, tag="t1", bufs=2)
                        nc.vector.tensor_mul(t1[:], sio[32:48, :],
                                             c_ps[:, sl])
                        a = ew.tile([BL, 512], F32, tag="a", bufs=2)
                        nc.vector.tensor_mul(a[:], sio[0:BL, :], tg[:])
                        nc.vector.tensor_add(c_ps[:, sl], t1[:], a[:])
                        z = ew.tile([80, 512], F32, tag="z", bufs=2)
                        nc.scalar.activation(z[64:80, :], c_ps[:, sl], AF.Tanh)
                        nc.vector.tensor_mul(h[:, sl], sio[64:80, :],
                                             z[64:80, :])
                        tr_half(h, nn, t)

        # ---------------- phase R1: encoder fwd + bwd ------------------------
        fwd_hT = hist.tile([128, 8, TOK], BF16, tag="hist")
        recurrence("rf", whh_f_d, xg_f, fwd_hT, reverse=False)
        bwd_hT = hist.tile([128, 8, TOK], BF16, tag="hist")
        recurrence("rb", whh_b_d, xg_b, bwd_hT, reverse=True)

        # ---------------- phase U: decoder input GEMM ------------------------
        # ug[tok] = encoded[tok] @ dec_w_ih^T + dec_b   (unshifted; read at t-1)
        with (
            tc.tile_pool(name="pu", bufs=1) as pu,
            tc.tile_pool(name="pu_w", bufs=3) as puw,
            tc.tile_pool(name="pu_ps", bufs=8, space="PSUM") as pups,
            tc.tile_pool(name="pu_ev", bufs=4) as puev,
        ):
            bias_sb = pu.tile([128, G4], F32, tag="biasd")
            nc.sync.dma_start(bias_sb[:], bias_d_d[:])
            n_mg = max(1, n_tok_blocks // 8)
            mg_w = min(8, n_tok_blocks)
            for n in range(8):
                for mg in range(n_mg):
                    pss = [pups.tile([tok_block, 512], F32, tag="ps",
                                     name=f"ps_{n}_{mg}_{i}")
                           for i in range(mg_w)]
                    for k in range(16):
                        wt = puw.tile([128, 512], BF16, tag="wt")
                        nc.sync.dma_start(wt[:], wih_d_d[k, :, n * 512:(n + 1) * 512])
                        src = fwd_hT if k < 8 else bwd_hT
                        for m in range(mg_w):
                            mb = mg * mg_w + m
                            nc.tensor.matmul(
                                pss[m][:],
                                src[:, k % 8, mb * tok_block:(mb + 1) * tok_block],
                                wt[:], start=(k == 0), stop=(k == 15))
                    for m in range(mg_w):
                        mb = mg * mg_w + m
                        ev = puev.tile([tok_block, 512], BF16, tag="ev")
                        nc.vector.tensor_add(
                            ev[:], pss[m][:],
                            bias_sb[:tok_block, n * 512:(n + 1) * 512])
                        nc.sync.dma_start(
                            ug_d[mb * tok_block:(mb + 1) * tok_block,
                                 n * 512:(n + 1) * 512], ev[:])

        # ---------------- phase R2: decoder recurrence -----------------------
        dec_hT = hist.tile([128, 8, TOK], BF16, tag="hist")
        recurrence("rd", whh_d_d, ug_d, dec_hT, reverse=False, dec_first=decb0_d)

        # ---------------- phase P: vocab projection --------------------------
        with (
            tc.tile_pool(name="pp", bufs=1) as pp,
            tc.tile_pool(name="pp_ps", bufs=2, space="PSUM") as ppps,
            tc.tile_pool(name="pp_ev", bufs=2) as ppev,
        ):
            wout_sb = pp.tile([128, 8, V], BF16, tag="wout")
            for k in range(8):
                nc.sync.dma_start(wout_sb[:, k, :], wout_d[k])
            outb_sb = pp.tile([V, 1], F32, tag="outb")
            nc.sync.dma_start(outb_sb[:], outb_d[:])
            for n in range(n_tok_chunks):
                ps = ppps.tile([V, tok_chunk], F32, tag="ps")
                for k in range(8):
                    nc.tensor.matmul(
                        ps[:], wout_sb[:, k, :],
                        dec_hT[:, k, n * tok_chunk:(n + 1) * tok_chunk],
                        start=(k == 0), stop=(k == 7))
                ev = ppev.tile([V, tok_chunk], F32, tag="ev")
                nc.vector.tensor_scalar_add(ev[:], ps[:], outb_sb[:])
                nc.sync.dma_start(pred_d[:, n * tok_chunk:(n + 1) * tok_chunk], ev[:])

    return nc


_CACHE = {}


def _get_nc(n_steps):
    if n_steps not in _CACHE:
        nc = bacc.Bacc("TRN2", target_bir_lowering=False, debug=False)
        _build(nc, n_steps)
        nc.compile()
        _CACHE[n_steps] = nc
    return _CACHE[n_steps]


def _gate_perm():
    r = np.arange(G4)
    return np.concatenate([r[0:H], r[2 * H:3 * H], r[H:2 * H], r[3 * H:4 * H]])


def _prep_shared(embedding, enc_w_ih_f, enc_w_hh_f, enc_b_f, enc_w_ih_b,
                 enc_w_hh_b, enc_b_b, dec_w_ih, dec_w_hh, dec_b, out_w, out_b):
    p = _gate_perm()

    def wT(w, kt):
        return np.ascontiguousarray(
            w[p].T.reshape(kt, 128, G4).astype(bf16_np))

    embp = np.zeros((VP, E), np.float32)
    embp[:V] = embedding
    shared = {
        "embp": embp.astype(bf16_np),
        "wih_f": wT(enc_w_ih_f, 4),
        "wih_b": wT(enc_w_ih_b, 4),
        "whh_f": wT(enc_w_hh_f, 8),
        "whh_b": wT(enc_w_hh_b, 8),
        "wih_d": wT(dec_w_ih, 16),
        "whh_d": wT(dec_w_hh, 8),
        "bias_f": np.broadcast_to(enc_b_f[p], (128, G4)).astype(np.float32).copy(),
        "bias_b": np.broadcast_to(enc_b_b[p], (128, G4)).astype(np.float32).copy(),
        "bias_d": np.broadcast_to(dec_b[p], (128, G4)).astype(np.float32).copy(),
        "decb0": np.broadcast_to(dec_b[p], (BL, G4)).astype(bf16_np).copy(),
        "wout": np.ascontiguousarray(
            out_w.T.reshape(8, 128, V).astype(bf16_np)),
        "outb": out_b.reshape(V, 1).astype(np.float32),
        "ident": np.eye(BL, dtype=bf16_np),
    }
    return shared


def _in_maps(inputs, n_steps):
    input_seq = np.asarray(inputs["input_seq"]).astype(np.int64)
    shared = _prep_shared(
        *[np.asarray(inputs[k], np.float32) for k in (
            "embedding", "enc_w_ih_f", "enc_w_hh_f", "enc_b_f",
            "enc_w_ih_b", "enc_w_hh_b", "enc_b_b",
            "dec_w_ih", "dec_w_hh", "dec_b", "out_w", "out_b")])
    TOK = n_steps * BL
    in_maps = []
    for c in range(NCORES):
        idx = input_seq[:n_steps, c * BL:(c + 1) * BL]  # [n_steps, BL]
        oh = np.zeros((VP, TOK), np.float32)
        cols = np.arange(TOK)
        oh[idx.reshape(-1), cols] = 1.0
        m = dict(shared)
        m["onehot"] = oh.astype(bf16_np)
        in_maps.append(m)
    return in_maps


def _assemble(res, n_steps):
    outs = []
    for c in range(NCORES):
        pr = res.results[c]["pred"]            # [V, TOK]
        outs.append(pr.reshape(V, n_steps, BL).transpose(1, 2, 0))
    return np.concatenate(outs, axis=1).astype(np.float32)  # [n_steps, B, V]


def _run(inputs, n_steps):
    in_maps = _in_maps(inputs, n_steps)
    nc = _get_nc(n_steps)
    res = run_bass_kernel_spmd(nc, in_maps, core_ids=list(range(NCORES)))
    return _assemble(res, n_steps)


def _register_ntff_hook():
    """Make antenv.axon_hooks importable (the image's antenv lacks it)."""
    import importlib.util
    if "antenv.axon_hooks" in sys.modules:
        return
    path = "/opt/trn_rl_repo/antenv/axon_hooks.py"
    if not os.path.exists(path):
        return
    spec = importlib.util.spec_from_file_location("antenv.axon_hooks", path)
    mod = importlib.util.module_from_spec(spec)
    spec.loader.exec_module(mod)
    sys.modules["antenv.axon_hooks"] = mod


def _run_traced(inputs, n_steps):
    _register_ntff_hook()
    in_maps = _in_maps(inputs, n_steps)
    nc = _get_nc(n_steps)
    res = run_bass_kernel_spmd(nc, in_maps, core_ids=list(range(NCORES)),
                               trace=True)
    return _assemble(res, n_steps), res


def kernel(**inputs):
    return _run(inputs, S)


# revision 28
# speedup vs baseline: 12.9066x; 1.0200x over previous
"""Trainium2 Bass kernel for nn_CharStemmer (bi-LSTM encoder + LSTM decoder).

Sharding: data-parallel over batch (B=128) across 8 cores, 16 sequences per
core; all weights replicated. Inside each core:
  - embedding lookup as one-hot matmul
  - input-side gate GEMMs (xg = emb @ w_ih^T + b) batched over all timesteps
  - the three recurrences run step-by-step; per step the stationary matmul
    operand is h^T (tiny) and w_hh^T streams through the PE in bf16
  - h is computed batch-major [16, 1024] for full-width elementwise, then
    PE-transposed into the hidden-major history h^T used as next-step lhsT
  - final vocab projection from the stored h^T history.
"""

import os
import sys

for _p in ("/opt/trn_rl_repo", "/root/.axon_site/_ro/trn_rl_repo"):
    if os.path.isdir(_p) and _p not in sys.path:
        sys.path.insert(0, _p)

from contextlib import ExitStack

import ml_dtypes
import numpy as np

import concourse.bass as bass
import concourse.tile as tile
from concourse import bacc, mybir
from concourse.bass_utils import run_bass_kernel_spmd

S, B, V, E, H = 128, 128, 61, 512, 1024
NCORES = 8
BL = B // NCORES          # 16 sequences per core
G4 = 4 * H                # 4096 gate columns
VP = 64                   # vocab padded to 64 partitions
BF16 = mybir.dt.bfloat16
F32 = mybir.dt.float32
AF = mybir.ActivationFunctionType
bf16_np = ml_dtypes.bfloat16

# gate quarters in permuted order: q0=i, q1=g, q2=f, q3=o


def _build(nc, n_steps):
    TOK = n_steps * BL

    def din(name, shape, dt):
        return nc.dram_tensor(name, list(shape), dt, kind="ExternalInput").ap()

    onehot_d = din("onehot", [VP, TOK], BF16)
    embp_d = din("embp", [VP, E], BF16)
    wih_f_d = din("wih_f", [4, 128, G4], BF16)
    wih_b_d = din("wih_b", [4, 128, G4], BF16)
    whh_f_d = din("whh_f", [8, 128, G4], BF16)
    whh_b_d = din("whh_b", [8, 128, G4], BF16)
    wih_d_d = din("wih_d", [16, 128, G4], BF16)
    whh_d_d = din("whh_d", [8, 128, G4], BF16)
    bias_f_d = din("bias_f", [128, G4], F32)
    bias_b_d = din("bias_b", [128, G4], F32)
    bias_d_d = din("bias_d", [128, G4], BF16)
    decb0_d = din("decb0", [BL, G4], BF16)   # decoder step-0 gates (bias only)
    wout_d = din("wout", [8, 128, V], BF16)
    outb_d = din("outb", [V, 1], F32)
    ident_d = din("ident", [BL, BL], BF16)
    pred_d = nc.dram_tensor("pred", [V, TOK], F32, kind="ExternalOutput").ap()

    with ExitStack() as ctx:
        tc = ctx.enter_context(tile.TileContext(nc))
        dram = ctx.enter_context(tc.tile_pool(name="dram", bufs=1, space="DRAM"))
        xg_f = dram.tile([TOK, G4], BF16, tag="xgf")
        xg_b = dram.tile([TOK, G4], BF16, tag="xgb")
        ug_d = dram.tile([TOK, G4], BF16, tag="ugd")

        persist = ctx.enter_context(tc.tile_pool(name="persist", bufs=1))

        ident_sb = persist.tile([BL, BL], BF16, tag="ident")
        nc.sync.dma_start(ident_sb[:], ident_d[:])

        # ---------------- phase E+X: embedding + input-side gate GEMMs -------
        n_tok_chunks = TOK // 512 if TOK >= 512 else 1
        tok_chunk = min(TOK, 512)
        n_tok_blocks = TOK // 128 if TOK >= 128 else 1
        tok_block = min(TOK, 128)

        with (
            tc.tile_pool(name="px", bufs=1) as px,
            tc.tile_pool(name="px_ps", bufs=8, space="PSUM") as px_ps,
            tc.tile_pool(name="px_ev", bufs=4) as px_ev,
        ):
            oh_sb = px.tile([VP, TOK], BF16, tag="oh")
            for j in range(TOK // 2048 if TOK >= 2048 else 1):
                w = min(TOK, 2048)
                nc.sync.dma_start(oh_sb[:, j * w:(j + 1) * w],
                                  onehot_d[:, j * w:(j + 1) * w])
            embp_sb = px.tile([VP, E], BF16, tag="embp")
            nc.sync.dma_start(embp_sb[:], embp_d[:])
            embT_sb = px.tile([128, 4 * TOK], BF16, tag="embT")

            # emb^T[e_chunk, tok] = embp.T @ onehot
            for m in range(4):
                for n in range(n_tok_chunks):
                    ps = px_ps.tile([128, tok_chunk], F32, tag="ps")
                    nc.tensor.matmul(
                        ps[:], embp_sb[:, m * 128:(m + 1) * 128],
                        oh_sb[:, n * tok_chunk:(n + 1) * tok_chunk],
                        start=True, stop=True)
                    nc.vector.tensor_copy(
                        embT_sb[:, m * TOK + n * tok_chunk:
                                m * TOK + (n + 1) * tok_chunk], ps[:])

            # xg = emb @ w_ih^T + b   (token-major [TOK, G4], f32, to DRAM)
            for wih_src, bias_src, xg_dst in (
                (wih_f_d, bias_f_d, xg_f),
                (wih_b_d, bias_b_d, xg_b),
            ):
                wih_sb = px.tile([128, 4, G4], BF16, tag="wih")
                for k in range(4):
                    nc.sync.dma_start(wih_sb[:, k, :], wih_src[k])
                bias_sb = px.tile([128, G4], F32, tag="bias")
                nc.sync.dma_start(bias_sb[:], bias_src[:])
                for m in range(n_tok_blocks):
                    for n in range(8):
                        ps = px_ps.tile([tok_block, 512], F32, tag="ps")
                        for k in range(4):
                            nc.tensor.matmul(
                                ps[:],
                                embT_sb[:, k * TOK + m * tok_block:
                                        k * TOK + (m + 1) * tok_block],
                                wih_sb[:, k, n * 512:(n + 1) * 512],
                                start=(k == 0), stop=(k == 3))
                        ev = px_ev.tile([tok_block, 512], BF16, tag="ev")
                        nc.vector.tensor_add(
                            ev[:], ps[:], bias_sb[:tok_block, n * 512:(n + 1) * 512])
                        nc.sync.dma_start(
                            xg_dst[m * tok_block:(m + 1) * tok_block,
                                   n * 512:(n + 1) * 512], ev[:])

        # ---------------- recurrence machinery -------------------------------
        # quarter order in xg cols is (i,g,f,o); col-group map: i->0 f->1 o->2 g->3
        QGRP = [0, 3, 1, 2]

        class LState:
            """Per-LSTM recurrence state. hT_tile: SBUF history [128,8,TOK]
            (decoder) or None when using a 2-slot ring + DRAM history (enc)."""

            def __init__(self, name, whh_sb, xg_src, reverse, pools, crow,
                         hT_tile=None, hT_dram=None, dec_first=None):
                self.name, self.whh_sb, self.xg = name, whh_sb, xg_src
                self.rev, self.crow = reverse, crow
                self.hT_tile, self.hT_dram = hT_tile, hT_dram
                self.dec_first = dec_first
                self.ew, self.xqp, self.qps, self.cps, self.trps, self.ringp \
                    = pools
                self.prev_ring = None

        def emit_step(L, s, n_steps):
            t = (n_steps - 1 - s) if L.rev else s
            tprev = (n_steps - s) if L.rev else (s - 1)
            nm = L.name
            xq = L.xqp.tile([BL, G4], BF16, tag="xq", name=f"xq_{nm}_{s}")
            if L.dec_first is not None and t == 0:
                nc.sync.dma_start(xq[:], L.dec_first[:])
            else:
                tsrc = (t - 1) if L.dec_first is not None else t
                nc.sync.dma_start(xq[:], L.xg[tsrc * BL:(tsrc + 1) * BL, :])

            crow = slice(L.crow, L.crow + BL)
            c_ps = L.cps
            if s > 0:
                ps = L.qps.tile([128, H], F32, tag="q", name=f"ps_{nm}_{s}")
                if L.hT_tile is not None:
                    def lhsT_at(k):
                        return L.hT_tile[:, k, tprev * BL:(tprev + 1) * BL]
                else:
                    ring_prev = L.prev_ring

                    def lhsT_at(k):
                        return ring_prev[:, k, :]
                for nn in range(2):
                    sl = slice(nn * 512, (nn + 1) * 512)
                    for q in range(4):
                        g = QGRP[q]
                        nc.tensor.matmul(
                            ps[32 * g:32 * g + 16, sl], ident_sb[:],
                            xq[:, q * H + nn * 512:q * H + (nn + 1) * 512],
                            start=True, stop=False, tile_position=(0, 32 * g))
                    for k in range(8):
                        lhsT = lhsT_at(k)
                        for q in range(4):
                            g = QGRP[q]
                            nc.tensor.matmul(
                                ps[32 * g:32 * g + 16, sl], lhsT,
                                L.whh_sb[:, k, q * H + nn * 512:
                                         q * H + (nn + 1) * 512],
                                start=False, stop=(k == 7),
                                tile_position=(0, 32 * g))

            h = L.ew.tile([BL, H], BF16, tag="h", bufs=2, name=f"h_{nm}_{s}")
            if L.hT_tile is None:
                ring = L.ringp.tile([128, 8, BL], BF16, tag="ring",
                                    name=f"ring_{nm}_{s}")
            for nn in range(2):
                sl = slice(nn * 512, (nn + 1) * 512)
                if s == 0:
                    # gates = xq only (h=c=0); quarters are xq col blocks
                    tg = L.ew.tile([BL, 512], F32, tag="tg", bufs=1,
                                   name=f"tg_{nm}_{s}_{nn}")
                    nc.scalar.activation(tg[:], xq[:, H + nn * 512:
                                                   H + (nn + 1) * 512], AF.Tanh)
                    si = L.ew.tile([BL, 512], F32, tag="sio", bufs=2,
                                   name=f"si_{nm}_{s}_{nn}")
                    nc.scalar.activation(si[:], xq[:, nn * 512:(nn + 1) * 512],
                                         AF.Sigmoid)
                    nc.vector.tensor_mul(c_ps[crow, sl], si[:], tg[:])
                    so = L.ew.tile([BL, 512], F32, tag="t1", bufs=1,
                                   name=f"so_{nm}_{s}_{nn}")
                    nc.scalar.activation(so[:], xq[:, 3 * H + nn * 512:
                                                   3 * H + (nn + 1) * 512],
                                         AF.Sigmoid)
                    z = L.ew.tile([BL, 512], F32, tag="a", bufs=1,
                                  name=f"z_{nm}_{s}_{nn}")
                    nc.scalar.activation(z[:], c_ps[crow, sl], AF.Tanh)
                    nc.vector.tensor_mul(h[:, sl], so[:], z[:])
                else:
                    tg = L.ew.tile([BL, 512], F32, tag="tg", bufs=1,
                                   name=f"tg_{nm}_{s}_{nn}")
                    nc.scalar.activation(tg[:], ps[96:112, sl], AF.Tanh)
                    sio = L.ew.tile([80, 512], F32, tag="sio", bufs=2,
                                    name=f"sio_{nm}_{s}_{nn}")
                    nc.scalar.activation(sio[:], ps[0:80, sl], AF.Sigmoid)
                    t1 = L.ew.tile([BL, 512], F32, tag="t1", bufs=1,
                                   name=f"t1_{nm}_{s}_{nn}")
                    nc.vector.tensor_mul(t1[:], sio[32:48, :], c_ps[crow, sl])
                    a = L.ew.tile([BL, 512], F32, tag="a", bufs=1,
                                  name=f"a_{nm}_{s}_{nn}")
                    nc.vector.tensor_mul(a[:], sio[0:BL, :], tg[:])
                    nc.vector.tensor_add(c_ps[crow, sl], t1[:], a[:])
                    z = L.ew.tile([80, 512], F32, tag="z", bufs=2,
                                  name=f"z_{nm}_{s}_{nn}")
                    nc.scalar.activation(z[64:80, :], c_ps[crow, sl], AF.Tanh)
                    nc.vector.tensor_mul(h[:, sl], sio[64:80, :], z[64:80, :])

                # transpose this half into h^T chunks nn*4..nn*4+3
                trb = L.trps.tile([128, 4 * BL], BF16, tag="tr",
                                  name=f"trb_{nm}_{s}_{nn}")
                for j in range(4):
                    kk = nn * 4 + j
                    nc.tensor.transpose(
                        trb[:, j * BL:(j + 1) * BL],
                        h[:, kk * 128:(kk + 1) * 128], ident_sb[:])
                trb_r = trb.rearrange("p (k b) -> p k b", k=4)
                ks = slice(nn * 4, (nn + 1) * 4)
                if L.hT_tile is not None:
                    nc.vector.tensor_copy(
                        L.hT_tile[:, ks, t * BL:(t + 1) * BL], trb_r[:, :, :])
                else:
                    nc.vector.tensor_copy(ring[:, ks, :], trb_r[:, :, :])
                    for j in range(4):
                        kk = nn * 4 + j
                        nc.sync.dma_start(
                            L.hT_dram[kk, :, t * BL:(t + 1) * BL],
                            ring[:, kk, :])
            if L.hT_tile is None:
                L.prev_ring = ring

        # ---------------- phase R1: encoder fwd + bwd interleaved ------------
        hTf_dram = dram.tile([8, 128, TOK], BF16, tag="hTf")
        hTb_dram = dram.tile([8, 128, TOK], BF16, tag="hTb")

        with (
            tc.tile_pool(name="enc_whh", bufs=1) as encw,
            tc.tile_pool(name="rf_ew", bufs=1) as few,
            tc.tile_pool(name="rb_ew", bufs=1) as bew,
            tc.tile_pool(name="r_xq", bufs=2) as xqp,
            tc.tile_pool(name="rf_q", bufs=1, space="PSUM") as fqps,
            tc.tile_pool(name="rb_q", bufs=1, space="PSUM") as bqps,
            tc.tile_pool(name="r_c", bufs=1, space="PSUM") as cpsp,
            tc.tile_pool(name="r_tr", bufs=2, space="PSUM") as trps,
            tc.tile_pool(name="r_ring", bufs=4) as ringp,
        ):
            whh_f_sb = encw.tile([128, 8, G4], BF16, tag="whhf",
                                 name="whh_f_sb")
            for k in range(8):
                nc.sync.dma_start(whh_f_sb[:, k, :], whh_f_d[k])
            whh_b_sb = encw.tile([128, 8, G4], BF16, tag="whhb",
                                 name="whh_b_sb")
            for k in range(8):
                nc.sync.dma_start(whh_b_sb[:, k, :], whh_b_d[k])
            c_sh = cpsp.tile([48, H], F32, tag="c")
            Lf = LState("rf", whh_f_sb, xg_f, False,
                        (few, xqp, fqps, c_sh, trps, ringp), 0,
                        hT_dram=hTf_dram)
            Lb = LState("rb", whh_b_sb, xg_b, True,
                        (bew, xqp, bqps, c_sh, trps, ringp), 32,
                        hT_dram=hTb_dram)
            for s in range(n_steps):
                emit_step(Lf, s, n_steps)
                emit_step(Lb, s, n_steps)

        # ---------------- phase U: decoder input GEMM ------------------------
        # ug[tok] = encoded[tok] @ dec_w_ih^T + dec_b   (unshifted; read at t-1)
        # History is restored from DRAM one token-half at a time (32KB slot).
        hist = ctx.enter_context(tc.tile_pool(name="hist", bufs=1))
        n_mg = max(1, n_tok_blocks // 8)
        mg_w = min(8, n_tok_blocks)
        mg_tok = mg_w * tok_block
        with (
            tc.tile_pool(name="pu", bufs=1) as pu,
            tc.tile_pool(name="pu_w", bufs=3) as puw,
            tc.tile_pool(name="pu_ps", bufs=8, space="PSUM") as pups,
            tc.tile_pool(name="pu_ev", bufs=4) as puev,
        ):
            bias_sb = pu.tile([128, G4], BF16, tag="biasd")
            nc.sync.dma_start(bias_sb[:], bias_d_d[:])
            for mg in range(n_mg):
                enc_hT = hist.tile([128, 16, mg_tok], BF16, tag="hist",
                                   name=f"enc_hT_{mg}")
                for k in range(16):
                    src = hTf_dram if k < 8 else hTb_dram
                    nc.sync.dma_start(
                        enc_hT[:, k, :],
                        src[k % 8][:, mg * mg_tok:(mg + 1) * mg_tok])
                for n in range(8):
                    pss = [pups.tile([tok_block, 512], F32, tag="ps",
                                     name=f"ps_{n}_{mg}_{i}")
                           for i in range(mg_w)]
                    for k in range(16):
                        wt = puw.tile([128, 512], BF16, tag="wt")
                        nc.sync.dma_start(
                            wt[:], wih_d_d[k, :, n * 512:(n + 1) * 512])
                        for m in range(mg_w):
                            nc.tensor.matmul(
                                pss[m][:],
                                enc_hT[:, k, m * tok_block:(m + 1) * tok_block],
                                wt[:], start=(k == 0), stop=(k == 15))
                    for m in range(mg_w):
                        mb = mg * mg_w + m
                        ev = puev.tile([tok_block, 512], BF16, tag="ev")
                        nc.vector.tensor_add(
                            ev[:], pss[m][:],
                            bias_sb[:tok_block, n * 512:(n + 1) * 512])
                        nc.sync.dma_start(
                            ug_d[mb * tok_block:(mb + 1) * tok_block,
                                 n * 512:(n + 1) * 512], ev[:])

        # ---------------- phase R2: decoder recurrence -----------------------
        dec_hT = hist.tile([128, 8, TOK], BF16, tag="hist")
        with (
            tc.tile_pool(name="dec_whh", bufs=1) as decw,
            tc.tile_pool(name="rd_ew", bufs=1) as dew,
            tc.tile_pool(name="rd_xq", bufs=3) as dxqp,
            tc.tile_pool(name="rd_q", bufs=2, space="PSUM") as dqps,
            tc.tile_pool(name="rd_c", bufs=1, space="PSUM") as dcpsp,
            tc.tile_pool(name="rd_tr", bufs=2, space="PSUM") as dtrps,
        ):
            whh_d_sb = decw.tile([128, 8, G4], BF16, tag="whhd",
                                 name="whh_d_sb")
            for k in range(8):
                nc.sync.dma_start(whh_d_sb[:, k, :], whh_d_d[k])
            c_d = dcpsp.tile([48, H], F32, tag="c")
            Ld = LState("rd", whh_d_sb, ug_d, False,
                        (dew, dxqp, dqps, c_d, dtrps, None), 0,
                        hT_tile=dec_hT, dec_first=decb0_d)
            for s in range(n_steps):
                emit_step(Ld, s, n_steps)

        # ---------------- phase P: vocab projection --------------------------
        with (
            tc.tile_pool(name="pp", bufs=1) as pp,
            tc.tile_pool(name="pp_ps", bufs=2, space="PSUM") as ppps,
            tc.tile_pool(name="pp_ev", bufs=2) as ppev,
        ):
            wout_sb = pp.tile([128, 8, V], BF16, tag="wout")
            for k in range(8):
                nc.sync.dma_start(wout_sb[:, k, :], wout_d[k])
            outb_sb = pp.tile([V, 1], F32, tag="outb")
            nc.sync.dma_start(outb_sb[:], outb_d[:])
            for n in range(n_tok_chunks):
                ps = ppps.tile([V, tok_chunk], F32, tag="ps")
                for k in range(8):
                    nc.tensor.matmul(
                        ps[:], wout_sb[:, k, :],
                        dec_hT[:, k, n * tok_chunk:(n + 1) * tok_chunk],
                        start=(k == 0), stop=(k == 7))
                ev = ppev.tile([V, tok_chunk], F32, tag="ev")
                nc.vector.tensor_scalar_add(ev[:], ps[:], outb_sb[:])
                nc.sync.dma_start(pred_d[:, n * tok_chunk:(n + 1) * tok_chunk], ev[:])

    return nc


_CACHE = {}


def _get_nc(n_steps):
    if n_steps not in _CACHE:
        nc = bacc.Bacc("TRN2", target_bir_lowering=False, debug=False)
        _build(nc, n_steps)
        nc.compile()
        _CACHE[n_steps] = nc
    return _CACHE[n_steps]


def _gate_perm():
    r = np.arange(G4)
    return np.concatenate([r[0:H], r[2 * H:3 * H], r[H:2 * H], r[3 * H:4 * H]])


def _prep_shared(embedding, enc_w_ih_f, enc_w_hh_f, enc_b_f, enc_w_ih_b,
                 enc_w_hh_b, enc_b_b, dec_w_ih, dec_w_hh, dec_b, out_w, out_b):
    p = _gate_perm()

    def wT(w, kt):
        return np.ascontiguousarray(
            w[p].T.reshape(kt, 128, G4).astype(bf16_np))

    embp = np.zeros((VP, E), np.float32)
    embp[:V] = embedding
    shared = {
        "embp": embp.astype(bf16_np),
        "wih_f": wT(enc_w_ih_f, 4),
        "wih_b": wT(enc_w_ih_b, 4),
        "whh_f": wT(enc_w_hh_f, 8),
        "whh_b": wT(enc_w_hh_b, 8),
        "wih_d": wT(dec_w_ih, 16),
        "whh_d": wT(dec_w_hh, 8),
        "bias_f": np.broadcast_to(enc_b_f[p], (128, G4)).astype(np.float32).copy(),
        "bias_b": np.broadcast_to(enc_b_b[p], (128, G4)).astype(np.float32).copy(),
        "bias_d": np.broadcast_to(dec_b[p], (128, G4)).astype(bf16_np).copy(),
        "decb0": np.broadcast_to(dec_b[p], (BL, G4)).astype(bf16_np).copy(),
        "wout": np.ascontiguousarray(
            out_w.T.reshape(8, 128, V).astype(bf16_np)),
        "outb": out_b.reshape(V, 1).astype(np.float32),
        "ident": np.eye(BL, dtype=bf16_np),
    }
    return shared


def _in_maps(inputs, n_steps):
    input_seq = np.asarray(inputs["input_seq"]).astype(np.int64)
    shared = _prep_shared(
        *[np.asarray(inputs[k], np.float32) for k in (
            "embedding", "enc_w_ih_f", "enc_w_hh_f", "enc_b_f",
            "enc_w_ih_b", "enc_w_hh_b", "enc_b_b",
            "dec_w_ih", "dec_w_hh", "dec_b", "out_w", "out_b")])
    TOK = n_steps * BL
    in_maps = []
    for c in range(NCORES):
        idx = input_seq[:n_steps, c * BL:(c + 1) * BL]  # [n_steps, BL]
        oh = np.zeros((VP, TOK), np.float32)
        cols = np.arange(TOK)
        oh[idx.reshape(-1), cols] = 1.0
        m = dict(shared)
        m["onehot"] = oh.astype(bf16_np)
        in_maps.append(m)
    return in_maps


def _assemble(res, n_steps):
    outs = []
    for c in range(NCORES):
        pr = res.results[c]["pred"]            # [V, TOK]
        outs.append(pr.reshape(V, n_steps, BL).transpose(1, 2, 0))
    return np.concatenate(outs, axis=1).astype(np.float32)  # [n_steps, B, V]


def _run(inputs, n_steps):
    in_maps = _in_maps(inputs, n_steps)
    nc = _get_nc(n_steps)
    res = run_bass_kernel_spmd(nc, in_maps, core_ids=list(range(NCORES)))
    return _assemble(res, n_steps)


def _register_ntff_hook():
    """Make antenv.axon_hooks importable (the image's antenv lacks it)."""
    import importlib.util
    if "antenv.axon_hooks" in sys.modules:
        return
    path = "/opt/trn_rl_repo/antenv/axon_hooks.py"
    if not os.path.exists(path):
        return
    spec = importlib.util.spec_from_file_location("antenv.axon_hooks", path)
    mod = importlib.util.module_from_spec(spec)
    spec.loader.exec_module(mod)
    sys.modules["antenv.axon_hooks"] = mod


def _run_traced(inputs, n_steps):
    _register_ntff_hook()
    in_maps = _in_maps(inputs, n_steps)
    nc = _get_nc(n_steps)
    res = run_bass_kernel_spmd(nc, in_maps, core_ids=list(range(NCORES)),
                               trace=True)
    return _assemble(res, n_steps), res


def kernel(**inputs):
    return _run(inputs, S)


# revision 30
# speedup vs baseline: 14.8132x; 1.1477x over previous
"""Trainium2 Bass kernel for nn_CharStemmer (bi-LSTM encoder + LSTM decoder).

Sharding: data-parallel over batch (B=128) across 8 cores, 16 sequences per
core; all weights replicated. Inside each core:
  - embedding lookup as one-hot matmul
  - input-side gate GEMMs (xg = emb @ w_ih^T + b) batched over all timesteps
  - the three recurrences run step-by-step; per step the stationary matmul
    operand is h^T (tiny) and w_hh^T streams through the PE in bf16
  - h is computed batch-major [16, 1024] for full-width elementwise, then
    PE-transposed into the hidden-major history h^T used as next-step lhsT
  - final vocab projection from the stored h^T history.
"""

import os
import sys

for _p in ("/opt/trn_rl_repo", "/root/.axon_site/_ro/trn_rl_repo"):
    if os.path.isdir(_p) and _p not in sys.path:
        sys.path.insert(0, _p)

from contextlib import ExitStack

import ml_dtypes
import numpy as np

import concourse.bass as bass
import concourse.tile as tile
from concourse import bacc, mybir
from concourse.bass_utils import run_bass_kernel_spmd

S, B, V, E, H = 128, 128, 61, 512, 1024
NCORES = 8
BL = B // NCORES          # 16 sequences per core
G4 = 4 * H                # 4096 gate columns
VP = 64                   # vocab padded to 64 partitions
BF16 = mybir.dt.bfloat16
F32 = mybir.dt.float32
AF = mybir.ActivationFunctionType
bf16_np = ml_dtypes.bfloat16

# gate quarters in permuted order: q0=i, q1=g, q2=f, q3=o


def _build(nc, n_steps):
    TOK = n_steps * BL

    def din(name, shape, dt):
        return nc.dram_tensor(name, list(shape), dt, kind="ExternalInput").ap()

    onehot_d = din("onehot", [VP, TOK], BF16)
    embp_d = din("embp", [VP, E], BF16)
    wih_f_d = din("wih_f", [4, 128, G4], BF16)
    wih_b_d = din("wih_b", [4, 128, G4], BF16)
    whh_f_d = din("whh_f", [8, 128, G4], BF16)
    whh_b_d = din("whh_b", [8, 128, G4], BF16)
    wih_d_d = din("wih_d", [16, 128, G4], BF16)
    whh_d_d = din("whh_d", [8, 128, G4], BF16)
    bias_f_d = din("bias_f", [128, G4], F32)
    bias_b_d = din("bias_b", [128, G4], F32)
    bias_d_d = din("bias_d", [128, G4], BF16)
    decb0_d = din("decb0", [BL, G4], BF16)   # decoder step-0 gates (bias only)
    wout_d = din("wout", [8, 128, V], BF16)
    outb_d = din("outb", [V, 1], F32)
    ident_d = din("ident", [BL, BL], BF16)
    pred_d = nc.dram_tensor("pred", [V, TOK], F32, kind="ExternalOutput").ap()

    with ExitStack() as ctx:
        tc = ctx.enter_context(tile.TileContext(nc))
        dram = ctx.enter_context(tc.tile_pool(name="dram", bufs=1, space="DRAM"))
        xg_f = dram.tile([TOK, G4], BF16, tag="xgf")
        xg_b = dram.tile([TOK, G4], BF16, tag="xgb")
        ug_d = dram.tile([TOK, G4], BF16, tag="ugd")

        persist = ctx.enter_context(tc.tile_pool(name="persist", bufs=1))

        ident_sb = persist.tile([BL, BL], BF16, tag="ident")
        nc.sync.dma_start(ident_sb[:], ident_d[:])

        # ---------------- phase E+X: embedding + input-side gate GEMMs -------
        n_tok_chunks = TOK // 512 if TOK >= 512 else 1
        tok_chunk = min(TOK, 512)
        n_tok_blocks = TOK // 128 if TOK >= 128 else 1
        tok_block = min(TOK, 128)

        with (
            tc.tile_pool(name="px", bufs=1) as px,
            tc.tile_pool(name="px_ps", bufs=8, space="PSUM") as px_ps,
            tc.tile_pool(name="px_ev", bufs=4) as px_ev,
        ):
            oh_sb = px.tile([VP, TOK], BF16, tag="oh")
            for j in range(TOK // 2048 if TOK >= 2048 else 1):
                w = min(TOK, 2048)
                nc.sync.dma_start(oh_sb[:, j * w:(j + 1) * w],
                                  onehot_d[:, j * w:(j + 1) * w])
            embp_sb = px.tile([VP, E], BF16, tag="embp")
            nc.sync.dma_start(embp_sb[:], embp_d[:])
            embT_sb = px.tile([128, 4 * TOK], BF16, tag="embT")

            # emb^T[e_chunk, tok] = embp.T @ onehot
            for m in range(4):
                for n in range(n_tok_chunks):
                    ps = px_ps.tile([128, tok_chunk], F32, tag="ps")
                    nc.tensor.matmul(
                        ps[:], embp_sb[:, m * 128:(m + 1) * 128],
                        oh_sb[:, n * tok_chunk:(n + 1) * tok_chunk],
                        start=True, stop=True)
                    nc.vector.tensor_copy(
                        embT_sb[:, m * TOK + n * tok_chunk:
                                m * TOK + (n + 1) * tok_chunk], ps[:])

            # xg = emb @ w_ih^T + b   (token-major [TOK, G4], f32, to DRAM)
            for wih_src, bias_src, xg_dst in (
                (wih_f_d, bias_f_d, xg_f),
                (wih_b_d, bias_b_d, xg_b),
            ):
                wih_sb = px.tile([128, 4, G4], BF16, tag="wih")
                for k in range(4):
                    nc.sync.dma_start(wih_sb[:, k, :], wih_src[k])
                bias_sb = px.tile([128, G4], F32, tag="bias")
                nc.sync.dma_start(bias_sb[:], bias_src[:])
                for m in range(n_tok_blocks):
                    for n in range(8):
                        ps = px_ps.tile([tok_block, 512], F32, tag="ps")
                        for k in range(4):
                            nc.tensor.matmul(
                                ps[:],
                                embT_sb[:, k * TOK + m * tok_block:
                                        k * TOK + (m + 1) * tok_block],
                                wih_sb[:, k, n * 512:(n + 1) * 512],
                                start=(k == 0), stop=(k == 3))
                        ev = px_ev.tile([tok_block, 512], BF16, tag="ev")
                        nc.vector.tensor_add(
                            ev[:], ps[:], bias_sb[:tok_block, n * 512:(n + 1) * 512])
                        nc.sync.dma_start(
                            xg_dst[m * tok_block:(m + 1) * tok_block,
                                   n * 512:(n + 1) * 512], ev[:])

        # ---------------- recurrence machinery -------------------------------
        # quarter order in xg cols is (i,g,f,o); col-group map: i->0 f->1 o->2 g->3
        QGRP = [0, 3, 1, 2]

        class LState:
            """Per-LSTM recurrence state. hT_tile: SBUF history [128,8,TOK]
            (decoder) or None when using a 2-slot ring + DRAM history (enc)."""

            def __init__(self, name, whh_sb, xg_src, reverse, pools, crow,
                         hT_tile=None, hT_dram=None, dec_first=None):
                self.name, self.whh_sb, self.xg = name, whh_sb, xg_src
                self.rev, self.crow = reverse, crow
                self.hT_tile, self.hT_dram = hT_tile, hT_dram
                self.dec_first = dec_first
                self.ew, self.xqp, self.qps, self.cps, self.trps, self.ringp \
                    = pools
                self.prev_ring = None

        def emit_step(L, s, n_steps):
            t = (n_steps - 1 - s) if L.rev else s
            tprev = (n_steps - s) if L.rev else (s - 1)
            nm = L.name
            xq = L.xqp.tile([BL, G4], BF16, tag="xq", name=f"xq_{nm}_{s}")
            if L.dec_first is not None and t == 0:
                nc.sync.dma_start(xq[:], L.dec_first[:])
            else:
                tsrc = (t - 1) if L.dec_first is not None else t
                nc.sync.dma_start(xq[:], L.xg[tsrc * BL:(tsrc + 1) * BL, :])

            crow = slice(L.crow, L.crow + BL)
            c_ps = L.cps
            pss = [None, None]
            if s > 0:
                if L.hT_tile is not None:
                    def lhsT_at(k):
                        return L.hT_tile[:, k, tprev * BL:(tprev + 1) * BL]
                else:
                    ring_prev = L.prev_ring

                    def lhsT_at(k):
                        return ring_prev[:, k, :]
                for nn in range(2):
                    ps = L.qps.tile([128, 512], F32, tag="q", bufs=2,
                                    name=f"ps_{nm}_{s}_{nn}")
                    pss[nn] = ps
                    for q in range(4):
                        g = QGRP[q]
                        nc.tensor.matmul(
                            ps[32 * g:32 * g + 16, :], ident_sb[:],
                            xq[:, q * H + nn * 512:q * H + (nn + 1) * 512],
                            start=True, stop=False, tile_position=(0, 32 * g))
                    for k in range(8):
                        lhsT = lhsT_at(k)
                        for q in range(4):
                            g = QGRP[q]
                            nc.tensor.matmul(
                                ps[32 * g:32 * g + 16, :], lhsT,
                                L.whh_sb[:, k, q * H + nn * 512:
                                         q * H + (nn + 1) * 512],
                                start=False, stop=(k == 7),
                                tile_position=(0, 32 * g))

            h = L.ew.tile([BL, H], BF16, tag="h", bufs=2, name=f"h_{nm}_{s}")
            if L.hT_tile is None:
                ring = L.ringp.tile([128, 8, BL], BF16, tag="ring",
                                    name=f"ring_{nm}_{s}")
            for nn in range(2):
                sl = slice(nn * 512, (nn + 1) * 512)
                if s == 0:
                    # gates = xq only (h=c=0); quarters are xq col blocks
                    tg = L.ew.tile([BL, 512], F32, tag="tg", bufs=1,
                                   name=f"tg_{nm}_{s}_{nn}")
                    nc.scalar.activation(tg[:], xq[:, H + nn * 512:
                                                   H + (nn + 1) * 512], AF.Tanh)
                    si = L.ew.tile([BL, 512], F32, tag="sio", bufs=2,
                                   name=f"si_{nm}_{s}_{nn}")
                    nc.scalar.activation(si[:], xq[:, nn * 512:(nn + 1) * 512],
                                         AF.Sigmoid)
                    nc.vector.tensor_mul(c_ps[crow, sl], si[:], tg[:])
                    so = L.ew.tile([BL, 512], F32, tag="t1", bufs=1,
                                   name=f"so_{nm}_{s}_{nn}")
                    nc.scalar.activation(so[:], xq[:, 3 * H + nn * 512:
                                                   3 * H + (nn + 1) * 512],
                                         AF.Sigmoid)
                    z = L.ew.tile([BL, 512], F32, tag="a", bufs=1,
                                  name=f"z_{nm}_{s}_{nn}")
                    nc.scalar.activation(z[:], c_ps[crow, sl], AF.Tanh)
                    nc.vector.tensor_mul(h[:, sl], so[:], z[:])
                else:
                    ps = pss[nn]
                    tg = L.ew.tile([BL, 512], F32, tag="tg", bufs=1,
                                   name=f"tg_{nm}_{s}_{nn}")
                    nc.scalar.activation(tg[:], ps[96:112, :], AF.Tanh)
                    sio = L.ew.tile([80, 512], F32, tag="sio", bufs=2,
                                    name=f"sio_{nm}_{s}_{nn}")
                    nc.scalar.activation(sio[:], ps[0:80, :], AF.Sigmoid)
                    t1 = L.ew.tile([BL, 512], F32, tag="t1", bufs=1,
                                   name=f"t1_{nm}_{s}_{nn}")
                    nc.vector.tensor_mul(t1[:], sio[32:48, :], c_ps[crow, sl])
                    a = L.ew.tile([BL, 512], F32, tag="a", bufs=1,
                                  name=f"a_{nm}_{s}_{nn}")
                    nc.vector.tensor_mul(a[:], sio[0:BL, :], tg[:])
                    nc.vector.tensor_add(c_ps[crow, sl], t1[:], a[:])
                    z = L.ew.tile([80, 512], F32, tag="z", bufs=2,
                                  name=f"z_{nm}_{s}_{nn}")
                    nc.scalar.activation(z[64:80, :], c_ps[crow, sl], AF.Tanh)
                    nc.vector.tensor_mul(h[:, sl], sio[64:80, :], z[64:80, :])

                # transpose this half into h^T chunks nn*4..nn*4+3
                trb = L.trps.tile([128, 4 * BL], BF16, tag="tr",
                                  name=f"trb_{nm}_{s}_{nn}")
                for j in range(4):
                    kk = nn * 4 + j
                    nc.tensor.transpose(
                        trb[:, j * BL:(j + 1) * BL],
                        h[:, kk * 128:(kk + 1) * 128], ident_sb[:])
                trb_r = trb.rearrange("p (k b) -> p k b", k=4)
                ks = slice(nn * 4, (nn + 1) * 4)
                if L.hT_tile is not None:
                    nc.vector.tensor_copy(
                        L.hT_tile[:, ks, t * BL:(t + 1) * BL], trb_r[:, :, :])
                else:
                    nc.vector.tensor_copy(ring[:, ks, :], trb_r[:, :, :])
                    for j in range(4):
                        kk = nn * 4 + j
                        nc.sync.dma_start(
                            L.hT_dram[kk, :, t * BL:(t + 1) * BL],
                            ring[:, kk, :])
            if L.hT_tile is None:
                L.prev_ring = ring

        # ---------------- phase R1: encoder fwd + bwd interleaved ------------
        hTf_dram = dram.tile([8, 128, TOK], BF16, tag="hTf")
        hTb_dram = dram.tile([8, 128, TOK], BF16, tag="hTb")

        with (
            tc.tile_pool(name="enc_whh", bufs=1) as encw,
            tc.tile_pool(name="rf_ew", bufs=1) as few,
            tc.tile_pool(name="rb_ew", bufs=1) as bew,
            tc.tile_pool(name="r_xq", bufs=2) as xqp,
            tc.tile_pool(name="rf_q", bufs=1, space="PSUM") as fqps,
            tc.tile_pool(name="rb_q", bufs=1, space="PSUM") as bqps,
            tc.tile_pool(name="r_c", bufs=1, space="PSUM") as cpsp,
            tc.tile_pool(name="r_tr", bufs=2, space="PSUM") as trps,
            tc.tile_pool(name="r_ring", bufs=4) as ringp,
        ):
            whh_f_sb = encw.tile([128, 8, G4], BF16, tag="whhf",
                                 name="whh_f_sb")
            for k in range(8):
                nc.sync.dma_start(whh_f_sb[:, k, :], whh_f_d[k])
            whh_b_sb = encw.tile([128, 8, G4], BF16, tag="whhb",
                                 name="whh_b_sb")
            for k in range(8):
                nc.sync.dma_start(whh_b_sb[:, k, :], whh_b_d[k])
            c_sh = cpsp.tile([48, H], F32, tag="c")
            Lf = LState("rf", whh_f_sb, xg_f, False,
                        (few, xqp, fqps, c_sh, trps, ringp), 0,
                        hT_dram=hTf_dram)
            Lb = LState("rb", whh_b_sb, xg_b, True,
                        (bew, xqp, bqps, c_sh, trps, ringp), 32,
                        hT_dram=hTb_dram)
            for s in range(n_steps):
                emit_step(Lf, s, n_steps)
                emit_step(Lb, s, n_steps)

        # ---------------- phase U: decoder input GEMM ------------------------
        # ug[tok] = encoded[tok] @ dec_w_ih^T + dec_b   (unshifted; read at t-1)
        # History is restored from DRAM one token-half at a time (32KB slot).
        hist = ctx.enter_context(tc.tile_pool(name="hist", bufs=1))
        n_mg = max(1, n_tok_blocks // 8)
        mg_w = min(8, n_tok_blocks)
        mg_tok = mg_w * tok_block
        with (
            tc.tile_pool(name="pu", bufs=1) as pu,
            tc.tile_pool(name="pu_w", bufs=3) as puw,
            tc.tile_pool(name="pu_ps", bufs=8, space="PSUM") as pups,
            tc.tile_pool(name="pu_ev", bufs=4) as puev,
        ):
            bias_sb = pu.tile([128, G4], BF16, tag="biasd")
            nc.sync.dma_start(bias_sb[:], bias_d_d[:])
            for mg in range(n_mg):
                enc_hT = hist.tile([128, 16, mg_tok], BF16, tag="hist",
                                   name=f"enc_hT_{mg}")
                for k in range(16):
                    src = hTf_dram if k < 8 else hTb_dram
                    nc.sync.dma_start(
                        enc_hT[:, k, :],
                        src[k % 8][:, mg * mg_tok:(mg + 1) * mg_tok])
                for n in range(8):
                    pss = [pups.tile([tok_block, 512], F32, tag="ps",
                                     name=f"ps_{n}_{mg}_{i}")
                           for i in range(mg_w)]
                    for k in range(16):
                        wt = puw.tile([128, 512], BF16, tag="wt")
                        nc.sync.dma_start(
                            wt[:], wih_d_d[k, :, n * 512:(n + 1) * 512])
                        for m in range(mg_w):
                            nc.tensor.matmul(
                                pss[m][:],
                                enc_hT[:, k, m * tok_block:(m + 1) * tok_block],
                                wt[:], start=(k == 0), stop=(k == 15))
                    for m in range(mg_w):
                        mb = mg * mg_w + m
                        ev = puev.tile([tok_block, 512], BF16, tag="ev")
                        nc.vector.tensor_add(
                            ev[:], pss[m][:],
                            bias_sb[:tok_block, n * 512:(n + 1) * 512])
                        nc.sync.dma_start(
                            ug_d[mb * tok_block:(mb + 1) * tok_block,
                                 n * 512:(n + 1) * 512], ev[:])

        # ---------------- phase R2: decoder recurrence -----------------------
        dec_hT = hist.tile([128, 8, TOK], BF16, tag="hist")
        with (
            tc.tile_pool(name="dec_whh", bufs=1) as decw,
            tc.tile_pool(name="rd_ew", bufs=1) as dew,
            tc.tile_pool(name="rd_xq", bufs=3) as dxqp,
            tc.tile_pool(name="rd_q", bufs=2, space="PSUM") as dqps,
            tc.tile_pool(name="rd_c", bufs=1, space="PSUM") as dcpsp,
            tc.tile_pool(name="rd_tr", bufs=2, space="PSUM") as dtrps,
        ):
            whh_d_sb = decw.tile([128, 8, G4], BF16, tag="whhd",
                                 name="whh_d_sb")
            for k in range(8):
                nc.sync.dma_start(whh_d_sb[:, k, :], whh_d_d[k])
            c_d = dcpsp.tile([48, H], F32, tag="c")
            Ld = LState("rd", whh_d_sb, ug_d, False,
                        (dew, dxqp, dqps, c_d, dtrps, None), 0,
                        hT_tile=dec_hT, dec_first=decb0_d)
            for s in range(n_steps):
                emit_step(Ld, s, n_steps)

        # ---------------- phase P: vocab projection --------------------------
        with (
            tc.tile_pool(name="pp", bufs=1) as pp,
            tc.tile_pool(name="pp_ps", bufs=2, space="PSUM") as ppps,
            tc.tile_pool(name="pp_ev", bufs=2) as ppev,
        ):
            wout_sb = pp.tile([128, 8, V], BF16, tag="wout")
            for k in range(8):
                nc.sync.dma_start(wout_sb[:, k, :], wout_d[k])
            outb_sb = pp.tile([V, 1], F32, tag="outb")
            nc.sync.dma_start(outb_sb[:], outb_d[:])
            for n in range(n_tok_chunks):
                ps = ppps.tile([V, tok_chunk], F32, tag="ps")
                for k in range(8):
                    nc.tensor.matmul(
                        ps[:], wout_sb[:, k, :],
                        dec_hT[:, k, n * tok_chunk:(n + 1) * tok_chunk],
                        start=(k == 0), stop=(k == 7))
                ev = ppev.tile([V, tok_chunk], F32, tag="ev")
                nc.vector.tensor_scalar_add(ev[:], ps[:], outb_sb[:])
                nc.sync.dma_start(pred_d[:, n * tok_chunk:(n + 1) * tok_chunk], ev[:])

    return nc


_CACHE = {}


def _get_nc(n_steps):
    if n_steps not in _CACHE:
        nc = bacc.Bacc("TRN2", target_bir_lowering=False, debug=False)
        _build(nc, n_steps)
        nc.compile()
        _CACHE[n_steps] = nc
    return _CACHE[n_steps]


def _gate_perm():
    r = np.arange(G4)
    return np.concatenate([r[0:H], r[2 * H:3 * H], r[H:2 * H], r[3 * H:4 * H]])


def _prep_shared(embedding, enc_w_ih_f, enc_w_hh_f, enc_b_f, enc_w_ih_b,
                 enc_w_hh_b, enc_b_b, dec_w_ih, dec_w_hh, dec_b, out_w, out_b):
    p = _gate_perm()

    def wT(w, kt):
        return np.ascontiguousarray(
            w[p].T.reshape(kt, 128, G4).astype(bf16_np))

    embp = np.zeros((VP, E), np.float32)
    embp[:V] = embedding
    shared = {
        "embp": embp.astype(bf16_np),
        "wih_f": wT(enc_w_ih_f, 4),
        "wih_b": wT(enc_w_ih_b, 4),
        "whh_f": wT(enc_w_hh_f, 8),
        "whh_b": wT(enc_w_hh_b, 8),
        "wih_d": wT(dec_w_ih, 16),
        "whh_d": wT(dec_w_hh, 8),
        "bias_f": np.broadcast_to(enc_b_f[p], (128, G4)).astype(np.float32).copy(),
        "bias_b": np.broadcast_to(enc_b_b[p], (128, G4)).astype(np.float32).copy(),
        "bias_d": np.broadcast_to(dec_b[p], (128, G4)).astype(bf16_np).copy(),
        "decb0": np.broadcast_to(dec_b[p], (BL, G4)).astype(bf16_np).copy(),
        "wout": np.ascontiguousarray(
            out_w.T.reshape(8, 128, V).astype(bf16_np)),
        "outb": out_b.reshape(V, 1).astype(np.float32),
        "ident": np.eye(BL, dtype=bf16_np),
    }
    return shared


def _in_maps(inputs, n_steps):
    input_seq = np.asarray(inputs["input_seq"]).astype(np.int64)
    shared = _prep_shared(
        *[np.asarray(inputs[k], np.float32) for k in (
            "embedding", "enc_w_ih_f", "enc_w_hh_f", "enc_b_f",
            "enc_w_ih_b", "enc_w_hh_b", "enc_b_b",
            "dec_w_ih", "dec_w_hh", "dec_b", "out_w", "out_b")])
    TOK = n_steps * BL
    in_maps = []
    for c in range(NCORES):
        idx = input_seq[:n_steps, c * BL:(c + 1) * BL]  # [n_steps, BL]
        oh = np.zeros((VP, TOK), np.float32)
        cols = np.arange(TOK)
        oh[idx.reshape(-1), cols] = 1.0
        m = dict(shared)
        m["onehot"] = oh.astype(bf16_np)
        in_maps.append(m)
    return in_maps


def _assemble(res, n_steps):
    outs = []
    for c in range(NCORES):
        pr = res.results[c]["pred"]            # [V, TOK]
        outs.append(pr.reshape(V, n_steps, BL).transpose(1, 2, 0))
    return np.concatenate(outs, axis=1).astype(np.float32)  # [n_steps, B, V]


def _run(inputs, n_steps):
    in_maps = _in_maps(inputs, n_steps)
    nc = _get_nc(n_steps)
    res = run_bass_kernel_spmd(nc, in_maps, core_ids=list(range(NCORES)))
    return _assemble(res, n_steps)


def _register_ntff_hook():
    """Make antenv.axon_hooks importable (the image's antenv lacks it)."""
    import importlib.util
    if "antenv.axon_hooks" in sys.modules:
        return
    path = "/opt/trn_rl_repo/antenv/axon_hooks.py"
    if not os.path.exists(path):
        return
    spec = importlib.util.spec_from_file_location("antenv.axon_hooks", path)
    mod = importlib.util.module_from_spec(spec)
    spec.loader.exec_module(mod)
    sys.modules["antenv.axon_hooks"] = mod


def _run_traced(inputs, n_steps):
    _register_ntff_hook()
    in_maps = _in_maps(inputs, n_steps)
    nc = _get_nc(n_steps)
    res = run_bass_kernel_spmd(nc, in_maps, core_ids=list(range(NCORES)),
                               trace=True)
    return _assemble(res, n_steps), res


def kernel(**inputs):
    return _run(inputs, S)


# revision 34
# speedup vs baseline: 14.8978x; 1.0057x over previous
"""Trainium2 Bass kernel for nn_CharStemmer (bi-LSTM encoder + LSTM decoder).

Sharding: data-parallel over batch (B=128) across 8 cores, 16 sequences per
core; all weights replicated. Inside each core:
  - embedding lookup as one-hot matmul
  - input-side gate GEMMs (xg = emb @ w_ih^T + b) batched over all timesteps
  - the three recurrences run step-by-step; per step the stationary matmul
    operand is h^T (tiny) and w_hh^T streams through the PE in bf16
  - h is computed batch-major [16, 1024] for full-width elementwise, then
    PE-transposed into the hidden-major history h^T used as next-step lhsT
  - final vocab projection from the stored h^T history.
"""

import os
import sys

for _p in ("/opt/trn_rl_repo", "/root/.axon_site/_ro/trn_rl_repo"):
    if os.path.isdir(_p) and _p not in sys.path:
        sys.path.insert(0, _p)

from contextlib import ExitStack

import ml_dtypes
import numpy as np

import concourse.bass as bass
import concourse.tile as tile
from concourse import bacc, mybir
from concourse.bass_utils import run_bass_kernel_spmd

S, B, V, E, H = 128, 128, 61, 512, 1024
NCORES = 8
BL = B // NCORES          # 16 sequences per core
G4 = 4 * H                # 4096 gate columns
VP = 64                   # vocab padded to 64 partitions
BF16 = mybir.dt.bfloat16
F32 = mybir.dt.float32
AF = mybir.ActivationFunctionType
bf16_np = ml_dtypes.bfloat16

# gate quarters in permuted order: q0=i, q1=g, q2=f, q3=o


def _build(nc, n_steps):
    TOK = n_steps * BL

    def din(name, shape, dt):
        return nc.dram_tensor(name, list(shape), dt, kind="ExternalInput").ap()

    onehot_d = din("onehot", [VP, TOK], BF16)
    embp_d = din("embp", [VP, E], BF16)
    wih_f_d = din("wih_f", [4, 128, G4], BF16)
    wih_b_d = din("wih_b", [4, 128, G4], BF16)
    whh_f_d = din("whh_f", [8, 128, G4], BF16)
    whh_b_d = din("whh_b", [8, 128, G4], BF16)
    wih_d_d = din("wih_d", [16, 128, G4], BF16)
    whh_d_d = din("whh_d", [8, 128, G4], BF16)
    bias_f_d = din("bias_f", [128, G4], F32)
    bias_b_d = din("bias_b", [128, G4], F32)
    bias_d_d = din("bias_d", [128, G4], BF16)
    decb0_d = din("decb0", [BL, G4], BF16)   # decoder step-0 gates (bias only)
    wout_d = din("wout", [8, 128, V], BF16)
    outb_d = din("outb", [V, 1], F32)
    ident_d = din("ident", [BL, BL], BF16)
    pred_d = nc.dram_tensor("pred", [V, TOK], F32, kind="ExternalOutput").ap()

    with ExitStack() as ctx:
        tc = ctx.enter_context(tile.TileContext(nc))
        dram = ctx.enter_context(tc.tile_pool(name="dram", bufs=1, space="DRAM"))
        xg_f = dram.tile([TOK, G4], BF16, tag="xgf")
        xg_b = dram.tile([TOK, G4], BF16, tag="xgb")
        n_ug = 2 if TOK >= 2048 else 1
        ug_halves = [dram.tile([TOK // n_ug, G4], BF16, tag=f"ugd{i}",
                               name=f"ug_d{i}") for i in range(n_ug)]

        persist = ctx.enter_context(tc.tile_pool(name="persist", bufs=1))

        ident_sb = persist.tile([BL, BL], BF16, tag="ident")
        nc.sync.dma_start(ident_sb[:], ident_d[:])

        # ---------------- phase E+X: embedding + input-side gate GEMMs -------
        n_tok_chunks = TOK // 512 if TOK >= 512 else 1
        tok_chunk = min(TOK, 512)
        n_tok_blocks = TOK // 128 if TOK >= 128 else 1
        tok_block = min(TOK, 128)

        with (
            tc.tile_pool(name="px", bufs=1) as px,
            tc.tile_pool(name="px_ps", bufs=8, space="PSUM") as px_ps,
            tc.tile_pool(name="px_ev", bufs=4) as px_ev,
        ):
            oh_sb = px.tile([VP, TOK], BF16, tag="oh")
            for j in range(TOK // 2048 if TOK >= 2048 else 1):
                w = min(TOK, 2048)
                nc.sync.dma_start(oh_sb[:, j * w:(j + 1) * w],
                                  onehot_d[:, j * w:(j + 1) * w])
            embp_sb = px.tile([VP, E], BF16, tag="embp")
            nc.sync.dma_start(embp_sb[:], embp_d[:])
            embT_sb = px.tile([128, 4 * TOK], BF16, tag="embT")

            # emb^T[e_chunk, tok] = embp.T @ onehot
            for m in range(4):
                for n in range(n_tok_chunks):
                    ps = px_ps.tile([128, tok_chunk], F32, tag="ps")
                    nc.tensor.matmul(
                        ps[:], embp_sb[:, m * 128:(m + 1) * 128],
                        oh_sb[:, n * tok_chunk:(n + 1) * tok_chunk],
                        start=True, stop=True)
                    nc.vector.tensor_copy(
                        embT_sb[:, m * TOK + n * tok_chunk:
                                m * TOK + (n + 1) * tok_chunk], ps[:])

            # xg = emb @ w_ih^T + b   (token-major [TOK, G4], f32, to DRAM)
            for wih_src, bias_src, xg_dst in (
                (wih_f_d, bias_f_d, xg_f),
                (wih_b_d, bias_b_d, xg_b),
            ):
                wih_sb = px.tile([128, 4, G4], BF16, tag="wih")
                for k in range(4):
                    nc.sync.dma_start(wih_sb[:, k, :], wih_src[k])
                bias_sb = px.tile([128, G4], F32, tag="bias")
                nc.sync.dma_start(bias_sb[:], bias_src[:])
                for m in range(n_tok_blocks):
                    for n in range(8):
                        ps = px_ps.tile([tok_block, 512], F32, tag="ps")
                        for k in range(4):
                            nc.tensor.matmul(
                                ps[:],
                                embT_sb[:, k * TOK + m * tok_block:
                                        k * TOK + (m + 1) * tok_block],
                                wih_sb[:, k, n * 512:(n + 1) * 512],
                                start=(k == 0), stop=(k == 3))
                        ev = px_ev.tile([tok_block, 512], BF16, tag="ev")
                        nc.vector.tensor_add(
                            ev[:], ps[:], bias_sb[:tok_block, n * 512:(n + 1) * 512])
                        nc.sync.dma_start(
                            xg_dst[m * tok_block:(m + 1) * tok_block,
                                   n * 512:(n + 1) * 512], ev[:])

        # ---------------- recurrence machinery -------------------------------
        # quarter order in xg cols is (i,g,f,o); col-group map: i->0 f->1 o->2 g->3
        QGRP = [0, 3, 1, 2]

        class LState:
            """Per-LSTM recurrence state. hT_tile: SBUF history [128,8,TOK]
            (decoder) or None when using a 2-slot ring + DRAM history (enc)."""

            def __init__(self, name, whh_sb, xg_src, reverse, pools, crow,
                         hT_tile=None, hT_dram=None, dec_first=None):
                self.name, self.whh_sb, self.xg = name, whh_sb, xg_src
                self.rev, self.crow = reverse, crow
                self.hT_tile, self.hT_dram = hT_tile, hT_dram
                self.dec_first = dec_first
                self.ew, self.xqp, self.qps, self.cps, self.trps, self.ringp \
                    = pools
                self.prev_ring = None

        def emit_step(L, s, n_steps):
            t = (n_steps - 1 - s) if L.rev else s
            tprev = (n_steps - s) if L.rev else (s - 1)
            nm = L.name
            xq = L.xqp.tile([BL, G4], BF16, tag="xq", name=f"xq_{nm}_{s}")
            if L.dec_first is not None and t == 0:
                nc.sync.dma_start(xq[:], L.dec_first[:])
            else:
                tsrc = (t - 1) if L.dec_first is not None else t
                src = L.xg
                if isinstance(src, list):
                    half = src[0].shape[0] // BL
                    src, tsrc = src[tsrc // half], tsrc % half
                nc.sync.dma_start(xq[:], src[tsrc * BL:(tsrc + 1) * BL, :])

            crow = slice(L.crow, L.crow + BL)
            c_ps = L.cps
            pss = [None, None]
            if s > 0:
                if L.hT_tile is not None:
                    def lhsT_at(k):
                        return L.hT_tile[:, k, tprev * BL:(tprev + 1) * BL]
                else:
                    ring_prev = L.prev_ring

                    def lhsT_at(k):
                        return ring_prev[:, k, :]
                for nn in range(2):
                    ps = L.qps.tile([128, 512], F32, tag="q", bufs=2,
                                    name=f"ps_{nm}_{s}_{nn}")
                    pss[nn] = ps
                    for q in range(4):
                        g = QGRP[q]
                        nc.tensor.matmul(
                            ps[32 * g:32 * g + 16, :], ident_sb[:],
                            xq[:, q * H + nn * 512:q * H + (nn + 1) * 512],
                            start=True, stop=False, tile_position=(0, 32 * g))
                    for k in range(8):
                        lhsT = lhsT_at(k)
                        for q in range(4):
                            g = QGRP[q]
                            nc.tensor.matmul(
                                ps[32 * g:32 * g + 16, :], lhsT,
                                L.whh_sb[:, k, q * H + nn * 512:
                                         q * H + (nn + 1) * 512],
                                start=False, stop=(k == 7),
                                tile_position=(0, 32 * g))

            h = L.ew.tile([BL, H], BF16, tag="h", bufs=2, name=f"h_{nm}_{s}")
            if L.hT_tile is None:
                ring = L.ringp.tile([128, 8, BL], BF16, tag="ring",
                                    name=f"ring_{nm}_{s}")
            for nn in range(2):
                sl = slice(nn * 512, (nn + 1) * 512)
                if s == 0:
                    # gates = xq only (h=c=0); quarters are xq col blocks
                    tg = L.ew.tile([BL, 512], F32, tag="tg", bufs=1,
                                   name=f"tg_{nm}_{s}_{nn}")
                    nc.scalar.activation(tg[:], xq[:, H + nn * 512:
                                                   H + (nn + 1) * 512], AF.Tanh)
                    si = L.ew.tile([BL, 512], F32, tag="sio", bufs=2,
                                   name=f"si_{nm}_{s}_{nn}")
                    nc.scalar.activation(si[:], xq[:, nn * 512:(nn + 1) * 512],
                                         AF.Sigmoid)
                    nc.vector.tensor_mul(c_ps[crow, sl], si[:], tg[:])
                    so = L.ew.tile([BL, 512], F32, tag="t1", bufs=1,
                                   name=f"so_{nm}_{s}_{nn}")
                    nc.scalar.activation(so[:], xq[:, 3 * H + nn * 512:
                                                   3 * H + (nn + 1) * 512],
                                         AF.Sigmoid)
                    z = L.ew.tile([BL, 512], F32, tag="a", bufs=1,
                                  name=f"z_{nm}_{s}_{nn}")
                    nc.scalar.activation(z[:], c_ps[crow, sl], AF.Tanh)
                    nc.vector.tensor_mul(h[:, sl], so[:], z[:])
                else:
                    ps = pss[nn]
                    tg = L.ew.tile([BL, 512], F32, tag="tg", bufs=1,
                                   name=f"tg_{nm}_{s}_{nn}")
                    nc.scalar.activation(tg[:], ps[96:112, :], AF.Tanh)
                    sio = L.ew.tile([80, 512], F32, tag="sio", bufs=2,
                                    name=f"sio_{nm}_{s}_{nn}")
                    nc.scalar.activation(sio[:], ps[0:80, :], AF.Sigmoid)
                    t1 = L.ew.tile([BL, 512], F32, tag="t1", bufs=1,
                                   name=f"t1_{nm}_{s}_{nn}")
                    nc.vector.tensor_mul(t1[:], sio[32:48, :], c_ps[crow, sl])
                    a = L.ew.tile([BL, 512], F32, tag="a", bufs=1,
                                  name=f"a_{nm}_{s}_{nn}")
                    nc.vector.tensor_mul(a[:], sio[0:BL, :], tg[:])
                    nc.vector.tensor_add(c_ps[crow, sl], t1[:], a[:])
                    z = L.ew.tile([80, 512], F32, tag="z", bufs=2,
                                  name=f"z_{nm}_{s}_{nn}")
                    nc.scalar.activation(z[64:80, :], c_ps[crow, sl], AF.Tanh)
                    nc.vector.tensor_mul(h[:, sl], sio[64:80, :], z[64:80, :])

                # transpose this half into h^T chunks nn*4..nn*4+3
                trb = L.trps.tile([128, 4 * BL], BF16, tag="tr",
                                  name=f"trb_{nm}_{s}_{nn}")
                for j in range(4):
                    kk = nn * 4 + j
                    nc.tensor.transpose(
                        trb[:, j * BL:(j + 1) * BL],
                        h[:, kk * 128:(kk + 1) * 128], ident_sb[:])
                trb_r = trb.rearrange("p (k b) -> p k b", k=4)
                ks = slice(nn * 4, (nn + 1) * 4)
                if L.hT_tile is not None:
                    nc.vector.tensor_copy(
                        L.hT_tile[:, ks, t * BL:(t + 1) * BL], trb_r[:, :, :])
                else:
                    nc.vector.tensor_copy(ring[:, ks, :], trb_r[:, :, :])
                    for j in range(4):
                        kk = nn * 4 + j
                        nc.sync.dma_start(
                            L.hT_dram[kk, :, t * BL:(t + 1) * BL],
                            ring[:, kk, :])
            if L.hT_tile is None:
                L.prev_ring = ring

        # ---------------- phase R1: encoder fwd + bwd interleaved ------------
        hTf_dram = dram.tile([8, 128, TOK], BF16, tag="hTf")
        hTb_dram = dram.tile([8, 128, TOK], BF16, tag="hTb")

        with (
            tc.tile_pool(name="enc_whh", bufs=1) as encw,
            tc.tile_pool(name="rf_ew", bufs=1) as few,
            tc.tile_pool(name="rb_ew", bufs=1) as bew,
            tc.tile_pool(name="r_xq", bufs=2) as xqp,
            tc.tile_pool(name="rf_q", bufs=1, space="PSUM") as fqps,
            tc.tile_pool(name="rb_q", bufs=1, space="PSUM") as bqps,
            tc.tile_pool(name="r_c", bufs=1, space="PSUM") as cpsp,
            tc.tile_pool(name="r_tr", bufs=2, space="PSUM") as trps,
            tc.tile_pool(name="r_ring", bufs=4) as ringp,
        ):
            whh_f_sb = encw.tile([128, 8, G4], BF16, tag="whhf",
                                 name="whh_f_sb")
            for k in range(8):
                nc.sync.dma_start(whh_f_sb[:, k, :], whh_f_d[k])
            whh_b_sb = encw.tile([128, 8, G4], BF16, tag="whhb",
                                 name="whh_b_sb")
            for k in range(8):
                nc.sync.dma_start(whh_b_sb[:, k, :], whh_b_d[k])
            c_sh = cpsp.tile([48, H], F32, tag="c")
            Lf = LState("rf", whh_f_sb, xg_f, False,
                        (few, xqp, fqps, c_sh, trps, ringp), 0,
                        hT_dram=hTf_dram)
            Lb = LState("rb", whh_b_sb, xg_b, True,
                        (bew, xqp, bqps, c_sh, trps, ringp), 32,
                        hT_dram=hTb_dram)
            for s in range(n_steps):
                emit_step(Lf, s, n_steps)
                emit_step(Lb, s, n_steps)

        # ---------------- phase U: decoder input GEMM ------------------------
        # ug[tok] = encoded[tok] @ dec_w_ih^T + dec_b   (unshifted; read at t-1)
        # History is restored from DRAM one token-half at a time (32KB slot).
        hist = ctx.enter_context(tc.tile_pool(name="hist", bufs=2))
        n_mg = max(1, n_tok_blocks // 8)
        mg_w = min(8, n_tok_blocks)
        mg_tok = mg_w * tok_block
        with (
            tc.tile_pool(name="pu", bufs=1) as pu,
            tc.tile_pool(name="pu_w", bufs=3) as puw,
            tc.tile_pool(name="pu_ps", bufs=8, space="PSUM") as pups,
            tc.tile_pool(name="pu_ev", bufs=4) as puev,
        ):
            bias_sb = pu.tile([128, G4], BF16, tag="biasd")
            nc.sync.dma_start(bias_sb[:], bias_d_d[:])
            for mg in range(n_mg):
                enc_hT = hist.tile([128, 16, mg_tok], BF16, tag="hist",
                                   name=f"enc_hT_{mg}")
                for k in range(16):
                    src = hTf_dram if k < 8 else hTb_dram
                    nc.sync.dma_start(
                        enc_hT[:, k, :],
                        src[k % 8][:, mg * mg_tok:(mg + 1) * mg_tok])
                for n in range(8):
                    pss = [pups.tile([tok_block, 512], F32, tag="ps",
                                     name=f"ps_{n}_{mg}_{i}")
                           for i in range(mg_w)]
                    for k in range(16):
                        wt = puw.tile([128, 512], BF16, tag="wt")
                        nc.sync.dma_start(
                            wt[:], wih_d_d[k, :, n * 512:(n + 1) * 512])
                        for m in range(mg_w):
                            nc.tensor.matmul(
                                pss[m][:],
                                enc_hT[:, k, m * tok_block:(m + 1) * tok_block],
                                wt[:], start=(k == 0), stop=(k == 15))
                    for m in range(mg_w):
                        mb = mg * mg_w + m
                        ev = puev.tile([tok_block, 512], BF16, tag="ev")
                        nc.vector.tensor_add(
                            ev[:], pss[m][:],
                            bias_sb[:tok_block, n * 512:(n + 1) * 512])
                        ug_dst = ug_halves[mb * tok_block * n_ug // TOK]
                        rb = (mb * tok_block) % (TOK // n_ug)
                        nc.sync.dma_start(
                            ug_dst[rb:rb + tok_block,
                                   n * 512:(n + 1) * 512], ev[:])

        # ---------------- phase R2: decoder recurrence -----------------------
        dec_hT = hist.tile([128, 8, TOK], BF16, tag="hist")
        with (
            tc.tile_pool(name="dec_whh", bufs=1) as decw,
            tc.tile_pool(name="rd_ew", bufs=1) as dew,
            tc.tile_pool(name="rd_xq", bufs=3) as dxqp,
            tc.tile_pool(name="rd_q", bufs=2, space="PSUM") as dqps,
            tc.tile_pool(name="rd_c", bufs=1, space="PSUM") as dcpsp,
            tc.tile_pool(name="rd_tr", bufs=2, space="PSUM") as dtrps,
        ):
            whh_d_sb = decw.tile([128, 8, G4], BF16, tag="whhd",
                                 name="whh_d_sb")
            for k in range(8):
                nc.sync.dma_start(whh_d_sb[:, k, :], whh_d_d[k])
            c_d = dcpsp.tile([48, H], F32, tag="c")
            Ld = LState("rd", whh_d_sb, ug_halves, False,
                        (dew, dxqp, dqps, c_d, dtrps, None), 0,
                        hT_tile=dec_hT, dec_first=decb0_d)
            for s in range(n_steps):
                emit_step(Ld, s, n_steps)

        # ---------------- phase P: vocab projection --------------------------
        with (
            tc.tile_pool(name="pp", bufs=1) as pp,
            tc.tile_pool(name="pp_ps", bufs=2, space="PSUM") as ppps,
            tc.tile_pool(name="pp_ev", bufs=2) as ppev,
        ):
            wout_sb = pp.tile([128, 8, V], BF16, tag="wout")
            for k in range(8):
                nc.sync.dma_start(wout_sb[:, k, :], wout_d[k])
            outb_sb = pp.tile([V, 1], F32, tag="outb")
            nc.sync.dma_start(outb_sb[:], outb_d[:])
            for n in range(n_tok_chunks):
                ps = ppps.tile([V, tok_chunk], F32, tag="ps")
                for k in range(8):
                    nc.tensor.matmul(
                        ps[:], wout_sb[:, k, :],
                        dec_hT[:, k, n * tok_chunk:(n + 1) * tok_chunk],
                        start=(k == 0), stop=(k == 7))
                ev = ppev.tile([V, tok_chunk], F32, tag="ev")
                nc.vector.tensor_scalar_add(ev[:], ps[:], outb_sb[:])
                nc.sync.dma_start(pred_d[:, n * tok_chunk:(n + 1) * tok_chunk], ev[:])

    return nc


_CACHE = {}


def _get_nc(n_steps):
    if n_steps not in _CACHE:
        nc = bacc.Bacc("TRN2", target_bir_lowering=False, debug=False)
        _build(nc, n_steps)
        nc.compile()
        _CACHE[n_steps] = nc
    return _CACHE[n_steps]


def _gate_perm():
    r = np.arange(G4)
    return np.concatenate([r[0:H], r[2 * H:3 * H], r[H:2 * H], r[3 * H:4 * H]])


def _prep_shared(embedding, enc_w_ih_f, enc_w_hh_f, enc_b_f, enc_w_ih_b,
                 enc_w_hh_b, enc_b_b, dec_w_ih, dec_w_hh, dec_b, out_w, out_b):
    p = _gate_perm()

    def wT(w, kt):
        return np.ascontiguousarray(
            w[p].T.reshape(kt, 128, G4).astype(bf16_np))

    embp = np.zeros((VP, E), np.float32)
    embp[:V] = embedding
    shared = {
        "embp": embp.astype(bf16_np),
        "wih_f": wT(enc_w_ih_f, 4),
        "wih_b": wT(enc_w_ih_b, 4),
        "whh_f": wT(enc_w_hh_f, 8),
        "whh_b": wT(enc_w_hh_b, 8),
        "wih_d": wT(dec_w_ih, 16),
        "whh_d": wT(dec_w_hh, 8),
        "bias_f": np.broadcast_to(enc_b_f[p], (128, G4)).astype(np.float32).copy(),
        "bias_b": np.broadcast_to(enc_b_b[p], (128, G4)).astype(np.float32).copy(),
        "bias_d": np.broadcast_to(dec_b[p], (128, G4)).astype(bf16_np).copy(),
        "decb0": np.broadcast_to(dec_b[p], (BL, G4)).astype(bf16_np).copy(),
        "wout": np.ascontiguousarray(
            out_w.T.reshape(8, 128, V).astype(bf16_np)),
        "outb": out_b.reshape(V, 1).astype(np.float32),
        "ident": np.eye(BL, dtype=bf16_np),
    }
    return shared


def _in_maps(inputs, n_steps):
    input_seq = np.asarray(inputs["input_seq"]).astype(np.int64)
    shared = _prep_shared(
        *[np.asarray(inputs[k], np.float32) for k in (
            "embedding", "enc_w_ih_f", "enc_w_hh_f", "enc_b_f",
            "enc_w_ih_b", "enc_w_hh_b", "enc_b_b",
            "dec_w_ih", "dec_w_hh", "dec_b", "out_w", "out_b")])
    TOK = n_steps * BL
    in_maps = []
    for c in range(NCORES):
        idx = input_seq[:n_steps, c * BL:(c + 1) * BL]  # [n_steps, BL]
        oh = np.zeros((VP, TOK), np.float32)
        cols = np.arange(TOK)
        oh[idx.reshape(-1), cols] = 1.0
        m = dict(shared)
        m["onehot"] = oh.astype(bf16_np)
        in_maps.append(m)
    return in_maps


def _assemble(res, n_steps):
    outs = []
    for c in range(NCORES):
        pr = res.results[c]["pred"]            # [V, TOK]
        outs.append(pr.reshape(V, n_steps, BL).transpose(1, 2, 0))
    return np.concatenate(outs, axis=1).astype(np.float32)  # [n_steps, B, V]


def _run(inputs, n_steps):
    in_maps = _in_maps(inputs, n_steps)
    nc = _get_nc(n_steps)
    res = run_bass_kernel_spmd(nc, in_maps, core_ids=list(range(NCORES)))
    return _assemble(res, n_steps)


def _register_ntff_hook():
    """Make antenv.axon_hooks importable (the image's antenv lacks it)."""
    import importlib.util
    if "antenv.axon_hooks" in sys.modules:
        return
    path = "/opt/trn_rl_repo/antenv/axon_hooks.py"
    if not os.path.exists(path):
        return
    spec = importlib.util.spec_from_file_location("antenv.axon_hooks", path)
    mod = importlib.util.module_from_spec(spec)
    spec.loader.exec_module(mod)
    sys.modules["antenv.axon_hooks"] = mod


def _run_traced(inputs, n_steps):
    _register_ntff_hook()
    in_maps = _in_maps(inputs, n_steps)
    nc = _get_nc(n_steps)
    res = run_bass_kernel_spmd(nc, in_maps, core_ids=list(range(NCORES)),
                               trace=True)
    return _assemble(res, n_steps), res


def kernel(**inputs):
    return _run(inputs, S)


# revision 35
# speedup vs baseline: 15.1349x; 1.0159x over previous
"""Trainium2 Bass kernel for nn_CharStemmer (bi-LSTM encoder + LSTM decoder).

Sharding: data-parallel over batch (B=128) across 8 cores, 16 sequences per
core; all weights replicated. Inside each core:
  - embedding lookup as one-hot matmul
  - input-side gate GEMMs (xg = emb @ w_ih^T + b) batched over all timesteps
  - the three recurrences run step-by-step; per step the stationary matmul
    operand is h^T (tiny) and w_hh^T streams through the PE in bf16
  - h is computed batch-major [16, 1024] for full-width elementwise, then
    PE-transposed into the hidden-major history h^T used as next-step lhsT
  - final vocab projection from the stored h^T history.
"""

import os
import sys

for _p in ("/opt/trn_rl_repo", "/root/.axon_site/_ro/trn_rl_repo"):
    if os.path.isdir(_p) and _p not in sys.path:
        sys.path.insert(0, _p)

from contextlib import ExitStack

import ml_dtypes
import numpy as np

import concourse.bass as bass
import concourse.tile as tile
from concourse import bacc, mybir
from concourse.bass_utils import run_bass_kernel_spmd

S, B, V, E, H = 128, 128, 61, 512, 1024
NCORES = 8
BL = B // NCORES          # 16 sequences per core
G4 = 4 * H                # 4096 gate columns
VP = 64                   # vocab padded to 64 partitions
BF16 = mybir.dt.bfloat16
F32 = mybir.dt.float32
AF = mybir.ActivationFunctionType
bf16_np = ml_dtypes.bfloat16

# gate quarters in permuted order: q0=i, q1=g, q2=f, q3=o


def _build(nc, n_steps):
    TOK = n_steps * BL

    def din(name, shape, dt):
        return nc.dram_tensor(name, list(shape), dt, kind="ExternalInput").ap()

    onehot_d = din("onehot", [VP, TOK], BF16)
    embp_d = din("embp", [VP, E], BF16)
    wih_f_d = din("wih_f", [4, 128, G4], BF16)
    wih_b_d = din("wih_b", [4, 128, G4], BF16)
    whh_f_d = din("whh_f", [8, 128, G4], BF16)
    whh_b_d = din("whh_b", [8, 128, G4], BF16)
    wih_d_d = din("wih_d", [16, 128, G4], BF16)
    whh_d_d = din("whh_d", [8, 128, G4], BF16)
    bias_f_d = din("bias_f", [128, G4], F32)
    bias_b_d = din("bias_b", [128, G4], F32)
    bias_d_d = din("bias_d", [128, G4], BF16)
    decb0_d = din("decb0", [BL, G4], BF16)   # decoder step-0 gates (bias only)
    wout_d = din("wout", [8, 128, V], BF16)
    outb_d = din("outb", [V, 1], F32)
    ident_d = din("ident", [BL, BL], BF16)
    pred_d = nc.dram_tensor("pred", [V, TOK], F32, kind="ExternalOutput").ap()

    with ExitStack() as ctx:
        tc = ctx.enter_context(tile.TileContext(nc))
        dram = ctx.enter_context(tc.tile_pool(name="dram", bufs=1, space="DRAM"))
        xg_f = dram.tile([TOK, G4], BF16, tag="xgf")
        xg_b = dram.tile([TOK, G4], BF16, tag="xgb")
        n_ug = 2 if TOK >= 2048 else 1
        ug_halves = [dram.tile([TOK // n_ug, G4], BF16, tag=f"ugd{i}",
                               name=f"ug_d{i}") for i in range(n_ug)]

        persist = ctx.enter_context(tc.tile_pool(name="persist", bufs=1))

        ident_sb = persist.tile([BL, BL], BF16, tag="ident")
        nc.sync.dma_start(ident_sb[:], ident_d[:])

        # ---------------- phase E+X: embedding + input-side gate GEMMs -------
        n_tok_chunks = TOK // 512 if TOK >= 512 else 1
        tok_chunk = min(TOK, 512)
        n_tok_blocks = TOK // 128 if TOK >= 128 else 1
        tok_block = min(TOK, 128)

        with (
            tc.tile_pool(name="px", bufs=1) as px,
            tc.tile_pool(name="px_ps", bufs=8, space="PSUM") as px_ps,
            tc.tile_pool(name="px_ev", bufs=4) as px_ev,
        ):
            oh_sb = px.tile([VP, TOK], BF16, tag="oh")
            for j in range(TOK // 2048 if TOK >= 2048 else 1):
                w = min(TOK, 2048)
                nc.sync.dma_start(oh_sb[:, j * w:(j + 1) * w],
                                  onehot_d[:, j * w:(j + 1) * w])
            embp_sb = px.tile([VP, E], BF16, tag="embp")
            nc.sync.dma_start(embp_sb[:], embp_d[:])
            embT_sb = px.tile([128, 4 * TOK], BF16, tag="embT")

            # emb^T[e_chunk, tok] = embp.T @ onehot
            for m in range(4):
                for n in range(n_tok_chunks):
                    ps = px_ps.tile([128, tok_chunk], F32, tag="ps")
                    nc.tensor.matmul(
                        ps[:], embp_sb[:, m * 128:(m + 1) * 128],
                        oh_sb[:, n * tok_chunk:(n + 1) * tok_chunk],
                        start=True, stop=True)
                    nc.vector.tensor_copy(
                        embT_sb[:, m * TOK + n * tok_chunk:
                                m * TOK + (n + 1) * tok_chunk], ps[:])

            # xg = emb @ w_ih^T + b   (token-major [TOK, G4], f32, to DRAM)
            for wih_src, bias_src, xg_dst in (
                (wih_f_d, bias_f_d, xg_f),
                (wih_b_d, bias_b_d, xg_b),
            ):
                wih_sb = px.tile([128, 4, G4], BF16, tag="wih")
                for k in range(4):
                    nc.sync.dma_start(wih_sb[:, k, :], wih_src[k])
                bias_sb = px.tile([128, G4], F32, tag="bias")
                nc.sync.dma_start(bias_sb[:], bias_src[:])
                for m in range(n_tok_blocks):
                    for n in range(8):
                        ps = px_ps.tile([tok_block, 512], F32, tag="ps")
                        for k in range(4):
                            nc.tensor.matmul(
                                ps[:],
                                embT_sb[:, k * TOK + m * tok_block:
                                        k * TOK + (m + 1) * tok_block],
                                wih_sb[:, k, n * 512:(n + 1) * 512],
                                start=(k == 0), stop=(k == 3))
                        ev = px_ev.tile([tok_block, 512], BF16, tag="ev")
                        nc.vector.tensor_add(
                            ev[:], ps[:], bias_sb[:tok_block, n * 512:(n + 1) * 512])
                        nc.sync.dma_start(
                            xg_dst[m * tok_block:(m + 1) * tok_block,
                                   n * 512:(n + 1) * 512], ev[:])

        # ---------------- recurrence machinery -------------------------------
        # quarter order in xg cols is (i,g,f,o); col-group map: i->0 f->1 o->2 g->3
        QGRP = [0, 3, 1, 2]

        class LState:
            """Per-LSTM recurrence state. hT_tile: SBUF history [128,8,TOK]
            (decoder) or None when using a 2-slot ring + DRAM history (enc)."""

            def __init__(self, name, whh_sb, xg_src, reverse, pools, crow,
                         hT_tile=None, hT_dram=None, dec_first=None):
                self.name, self.whh_sb, self.xg = name, whh_sb, xg_src
                self.rev, self.crow = reverse, crow
                self.hT_tile, self.hT_dram = hT_tile, hT_dram
                self.dec_first = dec_first
                self.ew, self.xqp, self.qps, self.cps, self.trps, self.ringp \
                    = pools
                self.prev_ring = None

        def emit_step(L, s, n_steps):
            t = (n_steps - 1 - s) if L.rev else s
            tprev = (n_steps - s) if L.rev else (s - 1)
            nm = L.name
            xq = L.xqp.tile([BL, G4], BF16, tag="xq", name=f"xq_{nm}_{s}")
            if L.dec_first is not None and t == 0:
                nc.sync.dma_start(xq[:], L.dec_first[:])
            else:
                tsrc = (t - 1) if L.dec_first is not None else t
                src = L.xg
                if isinstance(src, list):
                    half = src[0].shape[0] // BL
                    src, tsrc = src[tsrc // half], tsrc % half
                nc.sync.dma_start(xq[:], src[tsrc * BL:(tsrc + 1) * BL, :])

            crow = slice(L.crow, L.crow + BL)
            c_ps = L.cps
            pss = [None, None]
            if s > 0:
                if L.hT_tile is not None:
                    def lhsT_at(k):
                        return L.hT_tile[:, k, tprev * BL:(tprev + 1) * BL]
                else:
                    ring_prev = L.prev_ring

                    def lhsT_at(k):
                        return ring_prev[:, k, :]
                for nn in range(2):
                    ps = L.qps.tile([128, 512], F32, tag="q", bufs=2,
                                    name=f"ps_{nm}_{s}_{nn}")
                    pss[nn] = ps
                    for q in range(4):
                        g = QGRP[q]
                        nc.tensor.matmul(
                            ps[32 * g:32 * g + 16, :], ident_sb[:],
                            xq[:, q * H + nn * 512:q * H + (nn + 1) * 512],
                            start=True, stop=False, tile_position=(0, 32 * g))
                    for k in range(8):
                        lhsT = lhsT_at(k)
                        for q in range(4):
                            g = QGRP[q]
                            nc.tensor.matmul(
                                ps[32 * g:32 * g + 16, :], lhsT,
                                L.whh_sb[:, k, q * H + nn * 512:
                                         q * H + (nn + 1) * 512],
                                start=False, stop=(k == 7),
                                tile_position=(0, 32 * g))

            h = L.ew.tile([BL, H], BF16, tag="h", bufs=2, name=f"h_{nm}_{s}")
            if L.hT_tile is None:
                ring = L.ringp.tile([128, 8, BL], BF16, tag="ring",
                                    name=f"ring_{nm}_{s}")
            for nn in range(2):
                sl = slice(nn * 512, (nn + 1) * 512)
                if s == 0:
                    # gates = xq only (h=c=0); quarters are xq col blocks
                    tg = L.ew.tile([BL, 512], F32, tag="tg", bufs=1,
                                   name=f"tg_{nm}_{s}_{nn}")
                    nc.scalar.activation(tg[:], xq[:, H + nn * 512:
                                                   H + (nn + 1) * 512], AF.Tanh)
                    si = L.ew.tile([BL, 512], F32, tag="sio", bufs=2,
                                   name=f"si_{nm}_{s}_{nn}")
                    nc.scalar.activation(si[:], xq[:, nn * 512:(nn + 1) * 512],
                                         AF.Sigmoid)
                    nc.vector.tensor_mul(c_ps[crow, sl], si[:], tg[:])
                    so = L.ew.tile([BL, 512], F32, tag="t1", bufs=1,
                                   name=f"so_{nm}_{s}_{nn}")
                    nc.scalar.activation(so[:], xq[:, 3 * H + nn * 512:
                                                   3 * H + (nn + 1) * 512],
                                         AF.Sigmoid)
                    z = L.ew.tile([BL, 512], F32, tag="a", bufs=1,
                                  name=f"z_{nm}_{s}_{nn}")
                    nc.scalar.activation(z[:], c_ps[crow, sl], AF.Tanh)
                    nc.vector.tensor_mul(h[:, sl], so[:], z[:])
                else:
                    ps = pss[nn]
                    tg = L.ew.tile([BL, 512], F32, tag="tg", bufs=1,
                                   name=f"tg_{nm}_{s}_{nn}")
                    nc.scalar.activation(tg[:], ps[96:112, :], AF.Tanh)
                    sio = L.ew.tile([80, 512], F32, tag="sio", bufs=2,
                                    name=f"sio_{nm}_{s}_{nn}")
                    nc.scalar.activation(sio[:], ps[0:80, :], AF.Sigmoid)
                    t1 = L.ew.tile([BL, 512], F32, tag="t1", bufs=1,
                                   name=f"t1_{nm}_{s}_{nn}")
                    nc.vector.tensor_mul(t1[:], sio[32:48, :], c_ps[crow, sl])
                    a = L.ew.tile([BL, 512], F32, tag="a", bufs=1,
                                  name=f"a_{nm}_{s}_{nn}")
                    nc.vector.tensor_mul(a[:], sio[0:BL, :], tg[:])
                    nc.vector.tensor_add(c_ps[crow, sl], t1[:], a[:])
                    z = L.ew.tile([80, 512], F32, tag="z", bufs=2,
                                  name=f"z_{nm}_{s}_{nn}")
                    nc.scalar.activation(z[64:80, :], c_ps[crow, sl], AF.Tanh)
                    nc.vector.tensor_mul(h[:, sl], sio[64:80, :], z[64:80, :])

                # transpose this half into h^T chunks nn*4..nn*4+3
                trb = L.trps.tile([128, 4 * BL], BF16, tag="tr",
                                  name=f"trb_{nm}_{s}_{nn}")
                for j in range(4):
                    kk = nn * 4 + j
                    nc.tensor.transpose(
                        trb[:, j * BL:(j + 1) * BL],
                        h[:, kk * 128:(kk + 1) * 128], ident_sb[:])
                trb_r = trb.rearrange("p (k b) -> p k b", k=4)
                ks = slice(nn * 4, (nn + 1) * 4)
                if L.hT_tile is not None:
                    nc.vector.tensor_copy(
                        L.hT_tile[:, ks, t * BL:(t + 1) * BL], trb_r[:, :, :])
                else:
                    nc.vector.tensor_copy(ring[:, ks, :], trb_r[:, :, :])
                    for j in range(4):
                        kk = nn * 4 + j
                        nc.sync.dma_start(
                            L.hT_dram[kk, :, t * BL:(t + 1) * BL],
                            ring[:, kk, :])
            if L.hT_tile is None:
                L.prev_ring = ring

        # ---------------- phase R1: encoder fwd + bwd interleaved ------------
        hTf_dram = dram.tile([8, 128, TOK], BF16, tag="hTf")
        hTb_dram = dram.tile([8, 128, TOK], BF16, tag="hTb")

        with (
            tc.tile_pool(name="enc_whh", bufs=1) as encw,
            tc.tile_pool(name="rf_ew", bufs=1) as few,
            tc.tile_pool(name="rb_ew", bufs=1) as bew,
            tc.tile_pool(name="r_xq", bufs=2) as xqp,
            tc.tile_pool(name="rf_q", bufs=1, space="PSUM") as fqps,
            tc.tile_pool(name="rb_q", bufs=1, space="PSUM") as bqps,
            tc.tile_pool(name="r_c", bufs=1, space="PSUM") as cpsp,
            tc.tile_pool(name="r_tr", bufs=2, space="PSUM") as trps,
            tc.tile_pool(name="r_ring", bufs=4) as ringp,
        ):
            whh_f_sb = encw.tile([128, 8, G4], BF16, tag="whhf",
                                 name="whh_f_sb")
            for k in range(8):
                nc.sync.dma_start(whh_f_sb[:, k, :], whh_f_d[k])
            whh_b_sb = encw.tile([128, 8, G4], BF16, tag="whhb",
                                 name="whh_b_sb")
            for k in range(8):
                nc.sync.dma_start(whh_b_sb[:, k, :], whh_b_d[k])
            c_sh = cpsp.tile([48, H], F32, tag="c")
            Lf = LState("rf", whh_f_sb, xg_f, False,
                        (few, xqp, fqps, c_sh, trps, ringp), 0,
                        hT_dram=hTf_dram)
            Lb = LState("rb", whh_b_sb, xg_b, True,
                        (bew, xqp, bqps, c_sh, trps, ringp), 32,
                        hT_dram=hTb_dram)
            for s in range(n_steps):
                emit_step(Lf, s, n_steps)
                emit_step(Lb, s, n_steps)

        # ---------------- phase U: decoder input GEMM ------------------------
        # ug[tok] = encoded[tok] @ dec_w_ih^T + dec_b   (unshifted; read at t-1)
        # History is restored from DRAM one token-half at a time (32KB slot).
        hist = ctx.enter_context(tc.tile_pool(name="hist", bufs=2))
        n_mg = max(1, n_tok_blocks // 8)
        mg_w = min(8, n_tok_blocks)
        mg_tok = mg_w * tok_block
        with (
            tc.tile_pool(name="pu", bufs=1) as pu,
            tc.tile_pool(name="pu_w", bufs=2) as puw,
            tc.tile_pool(name="pu_ps", bufs=2, space="PSUM") as pups,
            tc.tile_pool(name="pu_ev", bufs=4) as puev,
        ):
            bias_sb = pu.tile([128, G4], BF16, tag="biasd")
            nc.sync.dma_start(bias_sb[:], bias_d_d[:])
            for mg in range(n_mg):
                enc_hT = hist.tile([128, 16, mg_tok], BF16, tag="hist",
                                   name=f"enc_hT_{mg}")
                for k in range(16):
                    src = hTf_dram if k < 8 else hTb_dram
                    nc.sync.dma_start(
                        enc_hT[:, k, :],
                        src[k % 8][:, mg * mg_tok:(mg + 1) * mg_tok])
                for n in range(8):
                    wt = puw.tile([128, 16, 512], BF16, tag="wt",
                                  name=f"wt_{mg}_{n}")
                    for k in range(16):
                        nc.sync.dma_start(
                            wt[:, k, :], wih_d_d[k, :, n * 512:(n + 1) * 512])
                    for m in range(mg_w):
                        ps = pups.tile([tok_block, 512], F32, tag="ps",
                                       name=f"ps_{n}_{mg}_{m}")
                        for k in range(16):
                            nc.tensor.matmul(
                                ps[:],
                                enc_hT[:, k, m * tok_block:(m + 1) * tok_block],
                                wt[:, k, :], start=(k == 0), stop=(k == 15))
                        mb = mg * mg_w + m
                        ev = puev.tile([tok_block, 512], BF16, tag="ev")
                        nc.vector.tensor_add(
                            ev[:], ps[:],
                            bias_sb[:tok_block, n * 512:(n + 1) * 512])
                        ug_dst = ug_halves[mb * tok_block * n_ug // TOK]
                        rb = (mb * tok_block) % (TOK // n_ug)
                        nc.sync.dma_start(
                            ug_dst[rb:rb + tok_block,
                                   n * 512:(n + 1) * 512], ev[:])

        # ---------------- phase R2: decoder recurrence -----------------------
        dec_hT = hist.tile([128, 8, TOK], BF16, tag="hist")
        with (
            tc.tile_pool(name="dec_whh", bufs=1) as decw,
            tc.tile_pool(name="rd_ew", bufs=1) as dew,
            tc.tile_pool(name="rd_xq", bufs=3) as dxqp,
            tc.tile_pool(name="rd_q", bufs=2, space="PSUM") as dqps,
            tc.tile_pool(name="rd_c", bufs=1, space="PSUM") as dcpsp,
            tc.tile_pool(name="rd_tr", bufs=2, space="PSUM") as dtrps,
        ):
            whh_d_sb = decw.tile([128, 8, G4], BF16, tag="whhd",
                                 name="whh_d_sb")
            for k in range(8):
                nc.sync.dma_start(whh_d_sb[:, k, :], whh_d_d[k])
            c_d = dcpsp.tile([48, H], F32, tag="c")
            Ld = LState("rd", whh_d_sb, ug_halves, False,
                        (dew, dxqp, dqps, c_d, dtrps, None), 0,
                        hT_tile=dec_hT, dec_first=decb0_d)
            for s in range(n_steps):
                emit_step(Ld, s, n_steps)

        # ---------------- phase P: vocab projection --------------------------
        with (
            tc.tile_pool(name="pp", bufs=1) as pp,
            tc.tile_pool(name="pp_ps", bufs=2, space="PSUM") as ppps,
            tc.tile_pool(name="pp_ev", bufs=2) as ppev,
        ):
            wout_sb = pp.tile([128, 8, V], BF16, tag="wout")
            for k in range(8):
                nc.sync.dma_start(wout_sb[:, k, :], wout_d[k])
            outb_sb = pp.tile([V, 1], F32, tag="outb")
            nc.sync.dma_start(outb_sb[:], outb_d[:])
            for n in range(n_tok_chunks):
                ps = ppps.tile([V, tok_chunk], F32, tag="ps")
                for k in range(8):
                    nc.tensor.matmul(
                        ps[:], wout_sb[:, k, :],
                        dec_hT[:, k, n * tok_chunk:(n + 1) * tok_chunk],
                        start=(k == 0), stop=(k == 7))
                ev = ppev.tile([V, tok_chunk], F32, tag="ev")
                nc.vector.tensor_scalar_add(ev[:], ps[:], outb_sb[:])
                nc.sync.dma_start(pred_d[:, n * tok_chunk:(n + 1) * tok_chunk], ev[:])

    return nc


_CACHE = {}


def _get_nc(n_steps):
    if n_steps not in _CACHE:
        nc = bacc.Bacc("TRN2", target_bir_lowering=False, debug=False)
        _build(nc, n_steps)
        nc.compile()
        _CACHE[n_steps] = nc
    return _CACHE[n_steps]


def _gate_perm():
    r = np.arange(G4)
    return np.concatenate([r[0:H], r[2 * H:3 * H], r[H:2 * H], r[3 * H:4 * H]])


def _prep_shared(embedding, enc_w_ih_f, enc_w_hh_f, enc_b_f, enc_w_ih_b,
                 enc_w_hh_b, enc_b_b, dec_w_ih, dec_w_hh, dec_b, out_w, out_b):
    p = _gate_perm()

    def wT(w, kt):
        return np.ascontiguousarray(
            w[p].T.reshape(kt, 128, G4).astype(bf16_np))

    embp = np.zeros((VP, E), np.float32)
    embp[:V] = embedding
    shared = {
        "embp": embp.astype(bf16_np),
        "wih_f": wT(enc_w_ih_f, 4),
        "wih_b": wT(enc_w_ih_b, 4),
        "whh_f": wT(enc_w_hh_f, 8),
        "whh_b": wT(enc_w_hh_b, 8),
        "wih_d": wT(dec_w_ih, 16),
        "whh_d": wT(dec_w_hh, 8),
        "bias_f": np.broadcast_to(enc_b_f[p], (128, G4)).astype(np.float32).copy(),
        "bias_b": np.broadcast_to(enc_b_b[p], (128, G4)).astype(np.float32).copy(),
        "bias_d": np.broadcast_to(dec_b[p], (128, G4)).astype(bf16_np).copy(),
        "decb0": np.broadcast_to(dec_b[p], (BL, G4)).astype(bf16_np).copy(),
        "wout": np.ascontiguousarray(
            out_w.T.reshape(8, 128, V).astype(bf16_np)),
        "outb": out_b.reshape(V, 1).astype(np.float32),
        "ident": np.eye(BL, dtype=bf16_np),
    }
    return shared


def _in_maps(inputs, n_steps):
    input_seq = np.asarray(inputs["input_seq"]).astype(np.int64)
    shared = _prep_shared(
        *[np.asarray(inputs[k], np.float32) for k in (
            "embedding", "enc_w_ih_f", "enc_w_hh_f", "enc_b_f",
            "enc_w_ih_b", "enc_w_hh_b", "enc_b_b",
            "dec_w_ih", "dec_w_hh", "dec_b", "out_w", "out_b")])
    TOK = n_steps * BL
    in_maps = []
    for c in range(NCORES):
        idx = input_seq[:n_steps, c * BL:(c + 1) * BL]  # [n_steps, BL]
        oh = np.zeros((VP, TOK), np.float32)
        cols = np.arange(TOK)
        oh[idx.reshape(-1), cols] = 1.0
        m = dict(shared)
        m["onehot"] = oh.astype(bf16_np)
        in_maps.append(m)
    return in_maps


def _assemble(res, n_steps):
    outs = []
    for c in range(NCORES):
        pr = res.results[c]["pred"]            # [V, TOK]
        outs.append(pr.reshape(V, n_steps, BL).transpose(1, 2, 0))
    return np.concatenate(outs, axis=1).astype(np.float32)  # [n_steps, B, V]


def _run(inputs, n_steps):
    in_maps = _in_maps(inputs, n_steps)
    nc = _get_nc(n_steps)
    res = run_bass_kernel_spmd(nc, in_maps, core_ids=list(range(NCORES)))
    return _assemble(res, n_steps)


def _register_ntff_hook():
    """Make antenv.axon_hooks importable (the image's antenv lacks it)."""
    import importlib.util
    if "antenv.axon_hooks" in sys.modules:
        return
    path = "/opt/trn_rl_repo/antenv/axon_hooks.py"
    if not os.path.exists(path):
        return
    spec = importlib.util.spec_from_file_location("antenv.axon_hooks", path)
    mod = importlib.util.module_from_spec(spec)
    spec.loader.exec_module(mod)
    sys.modules["antenv.axon_hooks"] = mod


def _run_traced(inputs, n_steps):
    _register_ntff_hook()
    in_maps = _in_maps(inputs, n_steps)
    nc = _get_nc(n_steps)
    res = run_bass_kernel_spmd(nc, in_maps, core_ids=list(range(NCORES)),
                               trace=True)
    return _assemble(res, n_steps), res


def kernel(**inputs):
    return _run(inputs, S)


# revision 39
# speedup vs baseline: 15.2441x; 1.0072x over previous
"""Trainium2 Bass kernel for nn_CharStemmer (bi-LSTM encoder + LSTM decoder).

Sharding: data-parallel over batch (B=128) across 8 cores, 16 sequences per
core; all weights replicated. Inside each core:
  - embedding lookup as one-hot matmul
  - input-side gate GEMMs (xg = emb @ w_ih^T + b) batched over all timesteps
  - the three recurrences run step-by-step; per step the stationary matmul
    operand is h^T (tiny) and w_hh^T streams through the PE in bf16
  - h is computed batch-major [16, 1024] for full-width elementwise, then
    PE-transposed into the hidden-major history h^T used as next-step lhsT
  - final vocab projection from the stored h^T history.
"""

import os
import sys

for _p in ("/opt/trn_rl_repo", "/root/.axon_site/_ro/trn_rl_repo"):
    if os.path.isdir(_p) and _p not in sys.path:
        sys.path.insert(0, _p)

from contextlib import ExitStack

import ml_dtypes
import numpy as np

import concourse.bass as bass
import concourse.tile as tile
from concourse import bacc, mybir
from concourse.bass_utils import run_bass_kernel_spmd

S, B, V, E, H = 128, 128, 61, 512, 1024
NCORES = 8
BL = B // NCORES          # 16 sequences per core
G4 = 4 * H                # 4096 gate columns
VP = 64                   # vocab padded to 64 partitions
BF16 = mybir.dt.bfloat16
F32 = mybir.dt.float32
AF = mybir.ActivationFunctionType
bf16_np = ml_dtypes.bfloat16

# gate quarters in permuted order: q0=i, q1=g, q2=f, q3=o


def _build(nc, n_steps):
    TOK = n_steps * BL

    def din(name, shape, dt):
        return nc.dram_tensor(name, list(shape), dt, kind="ExternalInput").ap()

    onehot_d = din("onehot", [VP, TOK], BF16)
    embp_d = din("embp", [VP, E], BF16)
    wih_f_d = din("wih_f", [4, 128, G4], BF16)
    wih_b_d = din("wih_b", [4, 128, G4], BF16)
    whh_f_d = din("whh_f", [8, 128, G4], BF16)
    whh_b_d = din("whh_b", [8, 128, G4], BF16)
    wih_d_d = din("wih_d", [16, 128, G4], BF16)
    whh_d_d = din("whh_d", [8, 128, G4], BF16)
    bias_f_d = din("bias_f", [128, G4], F32)
    bias_b_d = din("bias_b", [128, G4], F32)
    bias_d_d = din("bias_d", [128, G4], BF16)
    decb0_d = din("decb0", [BL, G4], BF16)   # decoder step-0 gates (bias only)
    wout_d = din("wout", [8, 128, V], BF16)
    outb_d = din("outb", [V, 1], F32)
    ident_d = din("ident", [BL, BL], BF16)
    pred_d = nc.dram_tensor("pred", [V, TOK], F32, kind="ExternalOutput").ap()

    with ExitStack() as ctx:
        tc = ctx.enter_context(tile.TileContext(nc))
        dram = ctx.enter_context(tc.tile_pool(name="dram", bufs=1, space="DRAM"))
        xg_f = dram.tile([TOK, G4], BF16, tag="xgf")
        xg_b = dram.tile([TOK, G4], BF16, tag="xgb")
        n_ugb = max(1, TOK // 128)
        ug_blocks = [dram.tile([min(TOK, 128), G4], BF16, tag=f"ugd{i}",
                               name=f"ug_d{i}") for i in range(n_ugb)]

        persist = ctx.enter_context(tc.tile_pool(name="persist", bufs=1))

        ident_sb = persist.tile([BL, BL], BF16, tag="ident")
        nc.sync.dma_start(ident_sb[:], ident_d[:])

        # ---------------- phase E+X: embedding + input-side gate GEMMs -------
        n_tok_chunks = TOK // 512 if TOK >= 512 else 1
        tok_chunk = min(TOK, 512)
        n_tok_blocks = TOK // 128 if TOK >= 128 else 1
        tok_block = min(TOK, 128)

        with (
            tc.tile_pool(name="px", bufs=1) as px,
            tc.tile_pool(name="px_ps", bufs=8, space="PSUM") as px_ps,
            tc.tile_pool(name="px_ev", bufs=4) as px_ev,
        ):
            oh_sb = px.tile([VP, TOK], BF16, tag="oh")
            for j in range(TOK // 2048 if TOK >= 2048 else 1):
                w = min(TOK, 2048)
                nc.sync.dma_start(oh_sb[:, j * w:(j + 1) * w],
                                  onehot_d[:, j * w:(j + 1) * w])
            embp_sb = px.tile([VP, E], BF16, tag="embp")
            nc.sync.dma_start(embp_sb[:], embp_d[:])
            embT_sb = px.tile([128, 4 * TOK], BF16, tag="embT")

            # emb^T[e_chunk, tok] = embp.T @ onehot
            for m in range(4):
                for n in range(n_tok_chunks):
                    ps = px_ps.tile([128, tok_chunk], F32, tag="ps")
                    nc.tensor.matmul(
                        ps[:], embp_sb[:, m * 128:(m + 1) * 128],
                        oh_sb[:, n * tok_chunk:(n + 1) * tok_chunk],
                        start=True, stop=True)
                    nc.vector.tensor_copy(
                        embT_sb[:, m * TOK + n * tok_chunk:
                                m * TOK + (n + 1) * tok_chunk], ps[:])

            # xg = emb @ w_ih^T + b   (token-major [TOK, G4], f32, to DRAM)
            for wih_src, bias_src, xg_dst in (
                (wih_f_d, bias_f_d, xg_f),
                (wih_b_d, bias_b_d, xg_b),
            ):
                wih_sb = px.tile([128, 4, G4], BF16, tag="wih")
                for k in range(4):
                    nc.sync.dma_start(wih_sb[:, k, :], wih_src[k])
                bias_sb = px.tile([128, G4], F32, tag="bias")
                nc.sync.dma_start(bias_sb[:], bias_src[:])
                for m in range(n_tok_blocks):
                    for n in range(8):
                        ps = px_ps.tile([tok_block, 512], F32, tag="ps")
                        for k in range(4):
                            nc.tensor.matmul(
                                ps[:],
                                embT_sb[:, k * TOK + m * tok_block:
                                        k * TOK + (m + 1) * tok_block],
                                wih_sb[:, k, n * 512:(n + 1) * 512],
                                start=(k == 0), stop=(k == 3))
                        ev = px_ev.tile([tok_block, 512], BF16, tag="ev")
                        nc.vector.tensor_add(
                            ev[:], ps[:], bias_sb[:tok_block, n * 512:(n + 1) * 512])
                        nc.sync.dma_start(
                            xg_dst[m * tok_block:(m + 1) * tok_block,
                                   n * 512:(n + 1) * 512], ev[:])

        # ---------------- recurrence machinery -------------------------------
        # quarter order in xg cols is (i,g,f,o); col-group map: i->0 f->1 o->2 g->3
        QGRP = [0, 3, 1, 2]

        class LState:
            """Per-LSTM recurrence state. hT_tile: SBUF history [128,8,TOK]
            (decoder) or None when using a 2-slot ring + DRAM history (enc)."""

            def __init__(self, name, whh_sb, xg_src, reverse, pools, crow,
                         hT_tile=None, hT_dram=None, dec_first=None):
                self.name, self.whh_sb, self.xg = name, whh_sb, xg_src
                self.rev, self.crow = reverse, crow
                self.hT_tile, self.hT_dram = hT_tile, hT_dram
                self.dec_first = dec_first
                self.ew, self.xqp, self.qps, self.cps, self.trps, self.ringp \
                    = pools
                self.prev_ring = None

        def emit_step(L, s, n_steps):
            t = (n_steps - 1 - s) if L.rev else s
            tprev = (n_steps - s) if L.rev else (s - 1)
            nm = L.name
            xq = L.xqp.tile([BL, G4], BF16, tag="xq", name=f"xq_{nm}_{s}")
            if L.dec_first is not None and t == 0:
                nc.sync.dma_start(xq[:], L.dec_first[:])
            else:
                tsrc = (t - 1) if L.dec_first is not None else t
                src = L.xg
                if isinstance(src, list):
                    blk = src[0].shape[0] // BL
                    src, tsrc = src[tsrc // blk], tsrc % blk
                nc.sync.dma_start(xq[:], src[tsrc * BL:(tsrc + 1) * BL, :])

            crow = slice(L.crow, L.crow + BL)
            c_ps = L.cps
            pss = [None, None]
            if s > 0:
                if L.hT_tile is not None:
                    def lhsT_at(k):
                        return L.hT_tile[:, k, tprev * BL:(tprev + 1) * BL]
                else:
                    ring_prev = L.prev_ring

                    def lhsT_at(k):
                        return ring_prev[:, k, :]
                for nn in range(2):
                    ps = L.qps.tile([128, 512], F32, tag="q", bufs=2,
                                    name=f"ps_{nm}_{s}_{nn}")
                    pss[nn] = ps
                    for q in range(4):
                        g = QGRP[q]
                        nc.tensor.matmul(
                            ps[32 * g:32 * g + 16, :], ident_sb[:],
                            xq[:, q * H + nn * 512:q * H + (nn + 1) * 512],
                            start=True, stop=False, tile_position=(0, 32 * g))
                    for k in range(8):
                        lhsT = lhsT_at(k)
                        for q in range(4):
                            g = QGRP[q]
                            nc.tensor.matmul(
                                ps[32 * g:32 * g + 16, :], lhsT,
                                L.whh_sb[:, k, q * H + nn * 512:
                                         q * H + (nn + 1) * 512],
                                start=False, stop=(k == 7),
                                tile_position=(0, 32 * g))

            h = L.ew.tile([BL, H], BF16, tag="h", bufs=2, name=f"h_{nm}_{s}")
            if L.hT_tile is None:
                ring = L.ringp.tile([128, 8, BL], BF16, tag="ring",
                                    name=f"ring_{nm}_{s}")
            for nn in range(2):
                sl = slice(nn * 512, (nn + 1) * 512)
                if s == 0:
                    # gates = xq only (h=c=0); quarters are xq col blocks
                    tg = L.ew.tile([BL, 512], F32, tag="tg", bufs=1,
                                   name=f"tg_{nm}_{s}_{nn}")
                    nc.scalar.activation(tg[:], xq[:, H + nn * 512:
                                                   H + (nn + 1) * 512], AF.Tanh)
                    si = L.ew.tile([BL, 512], F32, tag="sio", bufs=2,
                                   name=f"si_{nm}_{s}_{nn}")
                    nc.scalar.activation(si[:], xq[:, nn * 512:(nn + 1) * 512],
                                         AF.Sigmoid)
                    nc.vector.tensor_mul(c_ps[crow, sl], si[:], tg[:])
                    so = L.ew.tile([BL, 512], F32, tag="t1", bufs=1,
                                   name=f"so_{nm}_{s}_{nn}")
                    nc.scalar.activation(so[:], xq[:, 3 * H + nn * 512:
                                                   3 * H + (nn + 1) * 512],
                                         AF.Sigmoid)
                    z = L.ew.tile([BL, 512], F32, tag="a", bufs=1,
                                  name=f"z_{nm}_{s}_{nn}")
                    nc.scalar.activation(z[:], c_ps[crow, sl], AF.Tanh)
                    nc.vector.tensor_mul(h[:, sl], so[:], z[:])
                else:
                    ps = pss[nn]
                    tg = L.ew.tile([BL, 512], F32, tag="tg", bufs=1,
                                   name=f"tg_{nm}_{s}_{nn}")
                    nc.scalar.activation(tg[:], ps[96:112, :], AF.Tanh)
                    sio = L.ew.tile([80, 512], F32, tag="sio", bufs=2,
                                    name=f"sio_{nm}_{s}_{nn}")
                    nc.scalar.activation(sio[:], ps[0:80, :], AF.Sigmoid)
                    t1 = L.ew.tile([BL, 512], F32, tag="t1", bufs=1,
                                   name=f"t1_{nm}_{s}_{nn}")
                    nc.vector.tensor_mul(t1[:], sio[32:48, :], c_ps[crow, sl])
                    a = L.ew.tile([BL, 512], F32, tag="a", bufs=1,
                                  name=f"a_{nm}_{s}_{nn}")
                    nc.vector.tensor_mul(a[:], sio[0:BL, :], tg[:])
                    nc.vector.tensor_add(c_ps[crow, sl], t1[:], a[:])
                    z = L.ew.tile([80, 512], F32, tag="z", bufs=2,
                                  name=f"z_{nm}_{s}_{nn}")
                    nc.scalar.activation(z[64:80, :], c_ps[crow, sl], AF.Tanh)
                    nc.vector.tensor_mul(h[:, sl], sio[64:80, :], z[64:80, :])

                # transpose this half into h^T chunks nn*4..nn*4+3
                trb = L.trps.tile([128, 4 * BL], BF16, tag="tr",
                                  name=f"trb_{nm}_{s}_{nn}")
                for j in range(4):
                    kk = nn * 4 + j
                    nc.tensor.transpose(
                        trb[:, j * BL:(j + 1) * BL],
                        h[:, kk * 128:(kk + 1) * 128], ident_sb[:])
                trb_r = trb.rearrange("p (k b) -> p k b", k=4)
                ks = slice(nn * 4, (nn + 1) * 4)
                if L.hT_tile is not None:
                    nc.vector.tensor_copy(
                        L.hT_tile[:, ks, t * BL:(t + 1) * BL], trb_r[:, :, :])
                else:
                    nc.vector.tensor_copy(ring[:, ks, :], trb_r[:, :, :])
                    for j in range(4):
                        kk = nn * 4 + j
                        nc.sync.dma_start(
                            L.hT_dram[kk, :, t * BL:(t + 1) * BL],
                            ring[:, kk, :])
            if L.hT_tile is None:
                L.prev_ring = ring

        # ---------------- phase R1: encoder fwd + bwd interleaved ------------
        hTf_dram = dram.tile([8, 128, TOK], BF16, tag="hTf")
        hTb_dram = dram.tile([8, 128, TOK], BF16, tag="hTb")

        with (
            tc.tile_pool(name="enc_whh", bufs=1) as encw,
            tc.tile_pool(name="rf_ew", bufs=1) as few,
            tc.tile_pool(name="rb_ew", bufs=1) as bew,
            tc.tile_pool(name="r_xq", bufs=2) as xqp,
            tc.tile_pool(name="rf_q", bufs=1, space="PSUM") as fqps,
            tc.tile_pool(name="rb_q", bufs=1, space="PSUM") as bqps,
            tc.tile_pool(name="r_c", bufs=1, space="PSUM") as cpsp,
            tc.tile_pool(name="r_tr", bufs=2, space="PSUM") as trps,
            tc.tile_pool(name="r_ring", bufs=4) as ringp,
        ):
            whh_f_sb = encw.tile([128, 8, G4], BF16, tag="whhf",
                                 name="whh_f_sb")
            for k in range(8):
                nc.sync.dma_start(whh_f_sb[:, k, :], whh_f_d[k])
            whh_b_sb = encw.tile([128, 8, G4], BF16, tag="whhb",
                                 name="whh_b_sb")
            for k in range(8):
                nc.sync.dma_start(whh_b_sb[:, k, :], whh_b_d[k])
            c_sh = cpsp.tile([48, H], F32, tag="c")
            Lf = LState("rf", whh_f_sb, xg_f, False,
                        (few, xqp, fqps, c_sh, trps, ringp), 0,
                        hT_dram=hTf_dram)
            Lb = LState("rb", whh_b_sb, xg_b, True,
                        (bew, xqp, bqps, c_sh, trps, ringp), 32,
                        hT_dram=hTb_dram)
            for s in range(n_steps):
                emit_step(Lf, s, n_steps)
                emit_step(Lb, s, n_steps)

        # ---------------- phase U: decoder input GEMM ------------------------
        # ug[tok] = encoded[tok] @ dec_w_ih^T + dec_b   (unshifted; read at t-1)
        # History is restored from DRAM one token-half at a time (32KB slot).
        hist = ctx.enter_context(tc.tile_pool(name="hist", bufs=2))
        n_mg = max(1, n_tok_blocks // 8)
        mg_w = min(8, n_tok_blocks)
        mg_tok = mg_w * tok_block
        with (
            tc.tile_pool(name="pu", bufs=1) as pu,
            tc.tile_pool(name="pu_w", bufs=1) as puw,
            tc.tile_pool(name="pu_ps", bufs=2, space="PSUM") as pups,
            tc.tile_pool(name="pu_ev", bufs=4) as puev,
            tc.tile_pool(name="dec_whh", bufs=1) as decw,
            tc.tile_pool(name="rd_ew", bufs=1) as dew,
            tc.tile_pool(name="rd_xq", bufs=3) as dxqp,
            tc.tile_pool(name="rd_q", bufs=2, space="PSUM") as dqps,
            tc.tile_pool(name="rd_c", bufs=1, space="PSUM") as dcpsp,
            tc.tile_pool(name="rd_tr", bufs=2, space="PSUM") as dtrps,
        ):
            bias_sb = pu.tile([128, G4], BF16, tag="biasd")
            nc.sync.dma_start(bias_sb[:], bias_d_d[:])
            wt_cur = [None]

            def ug_unit(mg, n, m):
                """One (n, m) unit of the ug GEMM for half mg; DMAs the
                n-chunk weights on m==0."""
                if m == 0:
                    wt_cur[0] = puw.tile([128, 16, 512], BF16, tag="wt",
                                         name=f"wt_{mg}_{n}")
                    for k in range(16):
                        nc.sync.dma_start(
                            wt_cur[0][:, k, :],
                            wih_d_d[k, :, n * 512:(n + 1) * 512])
                enc_hT = enc_hTs[mg]
                ps = pups.tile([tok_block, 512], F32, tag="ps",
                               name=f"ps_{n}_{mg}_{m}")
                for k in range(16):
                    nc.tensor.matmul(
                        ps[:], enc_hT[:, k, m * tok_block:(m + 1) * tok_block],
                        wt_cur[0][:, k, :], start=(k == 0), stop=(k == 15))
                mb = mg * mg_w + m
                ev = puev.tile([tok_block, 512], BF16, tag="ev")
                nc.vector.tensor_add(
                    ev[:], ps[:], bias_sb[:tok_block, n * 512:(n + 1) * 512])
                nc.sync.dma_start(ug_blocks[mb][:, n * 512:(n + 1) * 512],
                                  ev[:])

            enc_hTs = {}

            def restore(mg):
                enc_hTs[mg] = hist.tile([128, 16, mg_tok], BF16, tag="hist",
                                        name=f"enc_hT_{mg}")
                for k in range(16):
                    src = hTf_dram if k < 8 else hTb_dram
                    nc.sync.dma_start(
                        enc_hTs[mg][:, k, :],
                        src[k % 8][:, mg * mg_tok:(mg + 1) * mg_tok])

            # first half of ug up-front; second half interleaved with decoder
            restore(0)
            for n in range(8):
                for m in range(mg_w):
                    ug_unit(0, n, m)
            units = []
            if n_mg > 1:
                restore(1)
                units = [(1, n, m) for n in range(8) for m in range(mg_w)]

            # ---------------- phase R2: decoder recurrence -------------------
            dec_hT = hist.tile([128, 8, TOK], BF16, tag="hist")
            whh_d_sb = decw.tile([128, 8, G4], BF16, tag="whhd",
                                 name="whh_d_sb")
            for k in range(8):
                nc.sync.dma_start(whh_d_sb[:, k, :], whh_d_d[k])
            c_d = dcpsp.tile([48, H], F32, tag="c")
            Ld = LState("rd", whh_d_sb, ug_blocks, False,
                        (dew, dxqp, dqps, c_d, dtrps, None), 0,
                        hT_tile=dec_hT, dec_first=decb0_d)
            for s in range(n_steps):
                emit_step(Ld, s, n_steps)
                if s >= 1 and units:
                    ug_unit(*units.pop(0))
            for u in units:
                ug_unit(*u)

        # ---------------- phase P: vocab projection --------------------------
        with (
            tc.tile_pool(name="pp", bufs=1) as pp,
            tc.tile_pool(name="pp_ps", bufs=2, space="PSUM") as ppps,
            tc.tile_pool(name="pp_ev", bufs=2) as ppev,
        ):
            wout_sb = pp.tile([128, 8, V], BF16, tag="wout")
            for k in range(8):
                nc.sync.dma_start(wout_sb[:, k, :], wout_d[k])
            outb_sb = pp.tile([V, 1], F32, tag="outb")
            nc.sync.dma_start(outb_sb[:], outb_d[:])
            for n in range(n_tok_chunks):
                ps = ppps.tile([V, tok_chunk], F32, tag="ps")
                for k in range(8):
                    nc.tensor.matmul(
                        ps[:], wout_sb[:, k, :],
                        dec_hT[:, k, n * tok_chunk:(n + 1) * tok_chunk],
                        start=(k == 0), stop=(k == 7))
                ev = ppev.tile([V, tok_chunk], F32, tag="ev")
                nc.vector.tensor_scalar_add(ev[:], ps[:], outb_sb[:])
                nc.sync.dma_start(pred_d[:, n * tok_chunk:(n + 1) * tok_chunk], ev[:])

    return nc


_CACHE = {}


def _get_nc(n_steps):
    if n_steps not in _CACHE:
        nc = bacc.Bacc("TRN2", target_bir_lowering=False, debug=False)
        _build(nc, n_steps)
        nc.compile()
        _CACHE[n_steps] = nc
    return _CACHE[n_steps]


def _gate_perm():
    r = np.arange(G4)
    return np.concatenate([r[0:H], r[2 * H:3 * H], r[H:2 * H], r[3 * H:4 * H]])


def _prep_shared(embedding, enc_w_ih_f, enc_w_hh_f, enc_b_f, enc_w_ih_b,
                 enc_w_hh_b, enc_b_b, dec_w_ih, dec_w_hh, dec_b, out_w, out_b):
    p = _gate_perm()

    def wT(w, kt):
        return np.ascontiguousarray(
            w[p].T.reshape(kt, 128, G4).astype(bf16_np))

    embp = np.zeros((VP, E), np.float32)
    embp[:V] = embedding
    shared = {
        "embp": embp.astype(bf16_np),
        "wih_f": wT(enc_w_ih_f, 4),
        "wih_b": wT(enc_w_ih_b, 4),
        "whh_f": wT(enc_w_hh_f, 8),
        "whh_b": wT(enc_w_hh_b, 8),
        "wih_d": wT(dec_w_ih, 16),
        "whh_d": wT(dec_w_hh, 8),
        "bias_f": np.broadcast_to(enc_b_f[p], (128, G4)).astype(np.float32).copy(),
        "bias_b": np.broadcast_to(enc_b_b[p], (128, G4)).astype(np.float32).copy(),
        "bias_d": np.broadcast_to(dec_b[p], (128, G4)).astype(bf16_np).copy(),
        "decb0": np.broadcast_to(dec_b[p], (BL, G4)).astype(bf16_np).copy(),
        "wout": np.ascontiguousarray(
            out_w.T.reshape(8, 128, V).astype(bf16_np)),
        "outb": out_b.reshape(V, 1).astype(np.float32),
        "ident": np.eye(BL, dtype=bf16_np),
    }
    return shared


def _in_maps(inputs, n_steps):
    input_seq = np.asarray(inputs["input_seq"]).astype(np.int64)
    shared = _prep_shared(
        *[np.asarray(inputs[k], np.float32) for k in (
            "embedding", "enc_w_ih_f", "enc_w_hh_f", "enc_b_f",
            "enc_w_ih_b", "enc_w_hh_b", "enc_b_b",
            "dec_w_ih", "dec_w_hh", "dec_b", "out_w", "out_b")])
    TOK = n_steps * BL
    in_maps = []
    for c in range(NCORES):
        idx = input_seq[:n_steps, c * BL:(c + 1) * BL]  # [n_steps, BL]
        oh = np.zeros((VP, TOK), np.float32)
        cols = np.arange(TOK)
        oh[idx.reshape(-1), cols] = 1.0
        m = dict(shared)
        m["onehot"] = oh.astype(bf16_np)
        in_maps.append(m)
    return in_maps


def _assemble(res, n_steps):
    outs = []
    for c in range(NCORES):
        pr = res.results[c]["pred"]            # [V, TOK]
        outs.append(pr.reshape(V, n_steps, BL).transpose(1, 2, 0))
    return np.concatenate(outs, axis=1).astype(np.float32)  # [n_steps, B, V]


def _run(inputs, n_steps):
    in_maps = _in_maps(inputs, n_steps)
    nc = _get_nc(n_steps)
    res = run_bass_kernel_spmd(nc, in_maps, core_ids=list(range(NCORES)))
    return _assemble(res, n_steps)


def _register_ntff_hook():
    """Make antenv.axon_hooks importable (the image's antenv lacks it)."""
    import importlib.util
    if "antenv.axon_hooks" in sys.modules:
        return
    path = "/opt/trn_rl_repo/antenv/axon_hooks.py"
    if not os.path.exists(path):
        return
    spec = importlib.util.spec_from_file_location("antenv.axon_hooks", path)
    mod = importlib.util.module_from_spec(spec)
    spec.loader.exec_module(mod)
    sys.modules["antenv.axon_hooks"] = mod


def _run_traced(inputs, n_steps):
    _register_ntff_hook()
    in_maps = _in_maps(inputs, n_steps)
    nc = _get_nc(n_steps)
    res = run_bass_kernel_spmd(nc, in_maps, core_ids=list(range(NCORES)),
                               trace=True)
    return _assemble(res, n_steps), res


def kernel(**inputs):
    return _run(inputs, S)
